# revision 21
# baseline (speedup 1.0000x reference)
"""Distributed single-head attention + MLP block for 8 TRN2 NeuronCores.

Reference computation (per batch b):
  Q = query @ Wq^T + bq ; K = key @ Wk^T + bk
  scores = Q @ K^T / sqrt(H) ; attn = softmax(scores)
  weighted = attn @ value + value
  h1 = relu(weighted @ Wo1^T + bo1)
  out = h1 @ Wo2^T + bo2 + weighted

Sharding: B=4 batches x 2 query-row halves = 8 shards. Each core gets its
1024 query rows plus the full 2048 keys/values of its batch; attention is
dense (non-causal) so no inter-core communication is needed.

Layout strategy: everything on-device lives feature-on-partitions
("T-layout", X^T[f, tok]) so all matmul contractions line up with zero
on-device transposes, and the host pre-packs every shard into the exact
[128, free] SBUF tiling the kernel consumes. All five GEMMs run in fp8
DoubleRow (2 contraction rows/cycle); fp8 weight tensors are scaled up
x64/x256 on the host to escape the e4m3 subnormal range and un-scaled for
free via the ScalarE activation's scale argument. The softmax needs no
max-subtraction: scores have std ~1/3 by construction.

Residual/bias algebra: the host ships vTb = value^T + bo2, so the kernel's
"weighted + bo2" residual costs nothing; feeding the MLP with w' = w + bo2
is corrected by bo1' = bo1 - Wo1 @ bo2 (exact), which lets the second MLP
GEMM skip ScalarE entirely (one DVE add straight out of PSUM).

PSUM is laid out as 2x rotating [128,1024] bank-pairs (scores/projection/
MLP accumulators, evacuated by ONE wide ScalarE activation each) + 3
single banks for the PV accumulation (processed in two ht-halves) + 1
norm/warmup bank. The softmax denominator is a 4-op contiguous DVE fold
tree over the [128, 16*512] exp tile, then the usual ones-matmul rowsum +
reciprocal + PE-broadcast. A dozen throwaway matmuls run during the
initial DMA wait to trip the PE HAM clock-gate to 2.4 GHz before the real
GEMM stream starts.
"""

import contextlib

import numpy as np
import ml_dtypes

import concourse.bass as bass
import concourse.mybir as mybir
import concourse.tile as tile
from concourse.bass_utils import run_bass_kernel_spmd

dt = mybir.dt
AF = mybir.ActivationFunctionType

H = 768          # model dim
B = 4            # batch
S = 2048         # sequence length
N_CORES = 8
QCHUNK = S * B // N_CORES        # 1024 query rows per core
HT = H // 128                    # 6 feature partition-tiles
KTILES = S // 128                # 16 key partition-tiles
QB = 512                         # q-block width (= PSUM bank, fp32)
NQB = QCHUNK // QB               # 2 q-blocks per core

FP8 = dt.float8e4
NP_FP8 = dt.np(FP8)
BF16 = dt.bfloat16
NP_BF16 = ml_dtypes.bfloat16
PMODE = mybir.MatmulPerfMode.DoubleRow

WQ_SCALE = 256.0                 # host premultiplier on Wq (incl 1/sqrt(H))
WK_SCALE = 64.0                  # host premultiplier on Wk
WO1_SCALE = 64.0                 # host premultiplier on Wo1
WO2_SCALE = 64.0                 # host premultiplier on Wo2


def build_kernel():
    nc = bass.Bass()

    qT_ext = nc.declare_dram_parameter("qT", [128, HT * QCHUNK], FP8, isOutput=False)
    kT_ext = nc.declare_dram_parameter("kT", [128, HT * S], FP8, isOutput=False)
    v_ext = nc.declare_dram_parameter("v", [128, KTILES * H], FP8, isOutput=False)
    vTb_ext = nc.declare_dram_parameter("vTb", [128, HT * QCHUNK], BF16,
                                        isOutput=False)
    w_ext = {
        name: nc.declare_dram_parameter(name, [128, HT * H], FP8, isOutput=False)
        for name in ("wqT", "wkT", "wo1T", "wo2T")
    }
    b_ext = nc.declare_dram_parameter("biases", [128, 3 * HT], dt.float32,
                                      isOutput=False)
    outT_ext = nc.declare_dram_parameter(
        "outT", [128, HT * QCHUNK], BF16, isOutput=True
    )

    with tile.TileContext(nc) as tc, nc.allow_low_precision(
        reason="fp8 matmul path is intentional; rel-err budget is 2e-2"
    ):
        _body(nc, tc, qT_ext, kT_ext, v_ext, vTb_ext, w_ext, b_ext, outT_ext)

    _split_multi_waits(nc)
    return nc


def _body(nc, tc, qT_ext, kT_ext, v_ext, vTb_ext, w_ext, b_ext, outT_ext):
    with contextlib.ExitStack() as ctx:
        const_pool = ctx.enter_context(tc.tile_pool(name="const", bufs=1))
        w_pool = ctx.enter_context(tc.tile_pool(name="w", bufs=1))
        act_pool = ctx.enter_context(tc.tile_pool(name="act", bufs=1))
        st_pool = ctx.enter_context(tc.tile_pool(name="st", bufs=1))
        out_pool = ctx.enter_context(tc.tile_pool(name="outs", bufs=3))
        # PSUM: 2 x [128,1024] rotating bank-pairs + 3 PV banks + 1 norm bank.
        ps_pair = ctx.enter_context(tc.tile_pool(name="ps_pair", bufs=2,
                                                 space="PSUM"))
        ps_one = ctx.enter_context(tc.tile_pool(name="ps_one", bufs=1,
                                                space="PSUM"))

        # ---- constants + PE warm-up (no DMA dependency: runs during the
        # input DMA head and trips the HAM clock gate to full rate) ----
        wu = const_pool.tile([128, 256], BF16, tag="warmup")
        nc.vector.memset(wu[:], 0.002)
        ones_f32 = const_pool.tile([128, 128], dt.float32, tag="ones_f32")
        nc.vector.memset(ones_f32[:], 1.0)
        ones_row = const_pool.tile([1, 128], dt.float32r, tag="ones_row")
        nc.vector.tensor_copy(ones_row[:], ones_f32[0:1, :])
        ones_col = const_pool.tile([128, 1], dt.float32r, tag="ones_col")
        nc.vector.tensor_copy(ones_col[:], ones_f32[:, 0:1])

        # ---- input DMAs in first-use order. The first-matmul chain (kT
        # block A, wkT chunk 0, biases, rest of wkT, kT block B) rides the
        # sync queue; ScalarE's queue opens with a dummy Ln activation that
        # pre-pays the ~2.7us ACT table load while those DMAs fly (Ln selects
        # natural_log_exp_and_others, which also holds Exp/Relu/Identity/
        # Copy, so this is the only table load in the kernel). ----
        actwarm = const_pool.tile([1, 2], dt.float32, tag="actwarm")
        nc.scalar.activation(actwarm[:], ones_f32[0:1, 0:2], AF.Ln)

        kT_in = act_pool.tile([128, HT * S], FP8, tag="kT_in")
        nc.sync.dma_start(kT_in[:, : HT * QB * 2], kT_ext[:, : HT * QB * 2])
        w_sb = {
            name: w_pool.tile([128, HT * H], FP8, tag=name, name=f"w_{name}")
            for name in ("wqT", "wkT", "wo1T", "wo2T")
        }
        nc.sync.dma_start(w_sb["wkT"][:, : HT * 128],
                          w_ext["wkT"][:, : HT * 128])
        nc.sync.dma_start(w_sb["wkT"][:, HT * 128:], w_ext["wkT"][:, HT * 128:])
        bias_sb = const_pool.tile([128, 3 * HT], dt.float32, tag="biases")
        nc.sync.dma_start(bias_sb[:], b_ext[:])
        biases = {name: bias_sb[:, i * HT:(i + 1) * HT]
                  for i, name in enumerate(("bq", "bk", "bo1"))}
        nc.sync.dma_start(kT_in[:, HT * QB * 2:], kT_ext[:, HT * QB * 2:])
        nc.scalar.dma_start(w_sb["wqT"][:], w_ext["wqT"][:])
        qT_in = act_pool.tile([128, HT * QCHUNK], FP8, tag="qT_in")
        nc.scalar.dma_start(qT_in[:], qT_ext[:])
        v_sb = act_pool.tile([128, KTILES * H], FP8, tag="v_in")
        nc.sync.dma_start(v_sb[:], v_ext[:])
        nc.sync.dma_start(w_sb["wo1T"][:], w_ext["wo1T"][:])
        nc.sync.dma_start(w_sb["wo2T"][:], w_ext["wo2T"][:])
        vTb_in = act_pool.tile([128, HT * QCHUNK], BF16, tag="vTb_in")
        nc.sync.dma_start(vTb_in[:], vTb_ext[:])

        # warm-up matmuls (~2.6us of junk PE work into the norm bank)
        for i in range(12):
            ps_wu = ps_one.tile([128, QB], dt.float32, tag="norm",
                                name=f"wu{i}")
            nc.tensor.matmul(ps_wu[:, :256], wu[:, :128], wu[:],
                             start=True, stop=True)

        def w3(name):
            return w_sb[name][:].rearrange("p (o t m) -> p (o t) m", o=HT, t=HT)

        # ---- K/Q projections: jo-contraction into [128,1024] bank-pairs,
        # one wide ScalarE activation per (ot, block-pair) ----
        def project(wname, bias, scale, x_in, nblocks, out_sb, out_col, tag):
            wv = w3(wname)
            xv = x_in[:].rearrange("p (n t q) -> p n t q", n=nblocks, t=HT)
            for nb2 in range(nblocks // 2):
                for ot in range(HT):
                    pair = ps_pair.tile([128, 2 * QB], dt.float32, tag="pair",
                                        name=f"ps_{tag}_{nb2}_{ot}")
                    for jo in range(HT // 2):
                        for h in range(2):
                            nc.tensor.matmul(
                                pair[:, h * QB:(h + 1) * QB],
                                wv[:, ot * HT + 2 * jo: ot * HT + 2 * jo + 2, :],
                                xv[:, 2 * nb2 + h, 2 * jo: 2 * jo + 2, :],
                                start=(jo == 0),
                                stop=(jo == HT // 2 - 1),
                                perf_mode=PMODE,
                            )
                    c0 = out_col(ot, nb2)
                    nc.scalar.activation(
                        out_sb[:, c0: c0 + 2 * QB], pair[:], AF.Identity,
                        bias=bias[:, ot: ot + 1], scale=scale,
                    )

        KT = act_pool.tile([128, HT * S], FP8, tag="KT", name="KT_full")
        project("wkT", biases["bk"], 1.0 / WK_SCALE, kT_in, 4, KT,
                lambda ot, nb2: ot * S + nb2 * 2 * QB, "KT")
        QT = act_pool.tile([128, HT * QCHUNK], FP8, tag="QT", name="proj_QT")
        project("wqT", biases["bq"], 1.0 / WQ_SCALE, qT_in, 2, QT,
                lambda ot, nb2: ot * QCHUNK + nb2 * 2 * QB, "QT")
        KT3 = KT[:].rearrange("p (t k) -> p t k", t=HT)
        QT3 = QT[:].rearrange("p (t q) -> p t q", t=HT)

        def vpair(jk, ht):
            """lhsT [128, 2, 128]: k-tile pair (2jk, 2jk+1), h-tile ht."""
            return (v_sb[:].rearrange("p (t h) -> p t h", t=KTILES)
                    [:, 2 * jk: 2 * jk + 2, ht * 128:(ht + 1) * 128])

        # ---- attention + MLP, software-pipelined across q-blocks ----
        state = {}

        def phase_scores(qb):
            """scores^T + exp, two k-tiles per PSUM bank-pair / ACTIVATE;
            the denominator accumulates on the (otherwise idle) DVE as
            incremental [128,1024] adds trailing the exps."""
            q0 = qb * QB
            expT = st_pool.tile([128, KTILES * QB], FP8, tag=f"expT{qb}",
                                name=f"expT{qb}")
            acc = st_pool.tile([128, 2 * QB], BF16, tag=f"acc{qb}",
                               name=f"acc{qb}")
            state[qb] = {"expT": expT}
            for p8 in range(KTILES // 2):
                pair = ps_pair.tile([128, 2 * QB], dt.float32, tag="pair",
                                    name=f"ps_s_{qb}_{p8}")
                for half in range(2):
                    kt = 2 * p8 + half
                    for jo in range(HT // 2):
                        nc.tensor.matmul(
                            pair[:, half * QB:(half + 1) * QB],
                            KT3[:, 2 * jo: 2 * jo + 2, kt * 128:(kt + 1) * 128],
                            QT3[:, 2 * jo: 2 * jo + 2, q0: q0 + QB],
                            start=(jo == 0),
                            stop=(jo == HT // 2 - 1),
                            perf_mode=PMODE,
                        )
                sl = expT[:, p8 * 2 * QB:(p8 + 1) * 2 * QB]
                nc.scalar.activation(sl, pair[:], AF.Exp)
                if p8 == 0:
                    nc.vector.tensor_copy(acc[:], sl)
                else:
                    nc.vector.tensor_add(acc[:], acc[:], sl)
            sum_part = st_pool.tile([128, QB], dt.float32r, tag=f"sump{qb}",
                                    name=f"sum_part{qb}")
            nc.vector.tensor_add(sum_part[:], acc[:, :QB], acc[:, QB:])
            state[qb]["sum_part"] = sum_part

        def phase_norm_ln(qb):
            """Partition-reduce the rowsum with one ones-matmul, then
            1/rowsum = exp(-ln(rowsum)) on ScalarE: no DVE reciprocal."""
            st = state[qb]
            ps_den = ps_one.tile([128, QB], dt.float32, tag="norm",
                                 name=f"ps_den{qb}")
            nc.tensor.matmul(ps_den[0:1, :], ones_col[:], st["sum_part"][:],
                             start=True, stop=True)
            logsum = st_pool.tile([1, QB], dt.float32r, tag="logsum",
                                  name=f"logsum{qb}")
            nc.scalar.activation(logsum[:], ps_den[0:1, :], AF.Ln)
            st["logsum"] = logsum

        def phase_norm_bcast(qb):
            st = state[qb]
            ps_b = ps_one.tile([128, QB], dt.float32, tag="norm",
                               name=f"ps_b{qb}")
            nc.tensor.matmul(ps_b[:], ones_row[:], st["logsum"][:],
                             start=True, stop=True)
            bcast = st_pool.tile([128, QB], dt.float32, tag="bcast",
                                 name=f"bcast{qb}")
            nc.scalar.activation(bcast[:], ps_b[:], AF.Exp, scale=-1.0)
            st["bcast"] = bcast

        def phase_pv_half(qb, half):
            """PV for 3 h-tiles over all 16 k-tiles; norm chain of this
            q-block interleaves under half 0."""
            st = state[qb]
            rhs8 = st["expT"][:].rearrange("p (j t q) -> p j t q",
                                           j=KTILES // 2, t=2)
            ps_w = [ps_one.tile([128, QB], dt.float32, tag=f"pvw{i}",
                                name=f"pvw{i}_{qb}_{half}")
                    for i in range(3)]
            for jk in range(KTILES // 2):
                if half == 0 and jk == 1:
                    phase_norm_ln(qb)
                if half == 0 and jk == 3:
                    phase_norm_bcast(qb)
                for i in range(3):
                    nc.tensor.matmul(
                        ps_w[i][:],
                        vpair(jk, 3 * half + i),
                        rhs8[:, jk],
                        start=(jk == 0),
                        stop=(jk == KTILES // 2 - 1),
                        perf_mode=PMODE,
                    )
            st[f"ps_w{half}"] = ps_w

        def phase_weighted_half(qb, half):
            """w = PV/rowsum + (value^T + bo2); bf16 residual + fp8 GEMM copy."""
            st = state[qb]
            ps_w = st[f"ps_w{half}"]
            if "wr" not in st:
                st["wr"] = st_pool.tile([128, HT * QB], BF16, tag=f"wr{qb}",
                                        name=f"wr{qb}")
                st["w8"] = st_pool.tile([128, HT * QB], FP8, tag=f"w8_{qb}",
                                        name=f"w8_{qb}")
            wr, w8 = st["wr"], st["w8"]
            for i in range(3):
                ht = 3 * half + i
                c0 = ht * QB
                tmp = out_pool.tile([128, QB], dt.float32, tag="wtmp",
                                    name=f"wtmp_{qb}_{ht}")
                nc.vector.tensor_mul(tmp[:], ps_w[i][:], st["bcast"][:])
                nc.vector.tensor_add(
                    wr[:, c0: c0 + QB], tmp[:],
                    vTb_in[:, qb * HT * QB + c0: qb * HT * QB + c0 + QB],
                )
                nc.scalar.copy(w8[:, c0: c0 + QB], wr[:, c0: c0 + QB])

        def phase_mlp_h1(qb):
            st = state[qb]
            w8v = st["w8"][:].rearrange("p (t q) -> p t q", t=HT)
            wv = w3("wo1T")
            h1 = st_pool.tile([128, HT * QB], FP8, tag=f"h1_{qb}",
                              name=f"h1T{qb}")
            for otp in range(HT // 2):
                pair = ps_pair.tile([128, 2 * QB], dt.float32, tag="pair",
                                    name=f"ps_h1_{qb}_{otp}")
                for h in range(2):
                    ot = 2 * otp + h
                    for jo in range(HT // 2):
                        nc.tensor.matmul(
                            pair[:, h * QB:(h + 1) * QB],
                            wv[:, ot * HT + 2 * jo: ot * HT + 2 * jo + 2, :],
                            w8v[:, 2 * jo: 2 * jo + 2, :],
                            start=(jo == 0),
                            stop=(jo == HT // 2 - 1),
                            perf_mode=PMODE,
                        )
                for h in range(2):
                    ot = 2 * otp + h
                    nc.scalar.activation(
                        h1[:, ot * QB:(ot + 1) * QB],
                        pair[:, h * QB:(h + 1) * QB],
                        AF.Relu, bias=biases["bo1"][:, ot: ot + 1],
                        scale=1.0 / WO1_SCALE,
                    )
            st["h1"] = h1

        def phase_mlp_out(qb):
            """out = h1 @ Wo2^T + (w + bo2): PSUM + residual in one DVE add,
            straight to bf16 DMA staging."""
            st = state[qb]
            h1v = st["h1"][:].rearrange("p (t q) -> p t q", t=HT)
            wv = w3("wo2T")
            for otp in range(HT // 2):
                pair = ps_pair.tile([128, 2 * QB], dt.float32, tag="pair",
                                    name=f"ps_o_{qb}_{otp}")
                for h in range(2):
                    ot = 2 * otp + h
                    for jo in range(HT // 2):
                        nc.tensor.matmul(
                            pair[:, h * QB:(h + 1) * QB],
                            wv[:, ot * HT + 2 * jo: ot * HT + 2 * jo + 2, :],
                            h1v[:, 2 * jo: 2 * jo + 2, :],
                            start=(jo == 0),
                            stop=(jo == HT // 2 - 1),
                            perf_mode=PMODE,
                        )
                o_mid = out_pool.tile([128, 2 * QB], BF16, tag="o_mid",
                                      name=f"omid_{qb}_{otp}")
                o_sb = out_pool.tile([128, 2 * QB], BF16, tag="outT_blk",
                                     name=f"outT_{qb}_{otp}")
                c0 = (qb * HT + otp * 2) * QB
                # the very last block streams out in 512-wide chunks so the
                # act -> add -> DMA tail pipeline overlaps
                nchunk = 2 if (qb, otp) == (1, HT // 2 - 1) else 1
                for ch in range(nchunk):
                    w = 2 * QB // nchunk
                    sl = slice(ch * w, (ch + 1) * w)
                    nc.scalar.activation(o_mid[:, sl], pair[:, sl], AF.Identity,
                                         scale=1.0 / WO2_SCALE)
                    nc.vector.tensor_add(
                        o_sb[:, sl], o_mid[:, sl],
                        st["wr"][:, otp * 2 * QB + ch * w:
                                 otp * 2 * QB + (ch + 1) * w],
                    )
                    nc.sync.dma_start(
                        outT_ext[:, c0 + ch * w: c0 + (ch + 1) * w],
                        o_sb[:, sl],
                    )

        # software pipeline: DVE/ScalarE chains (norm, weighted, h1-acts) are
        # always covered by an independent PE phase emitted around them
        phase_scores(0)
        phase_pv_half(0, 0)
        phase_weighted_half(0, 0)
        phase_pv_half(0, 1)
        phase_weighted_half(0, 1)
        phase_scores(1)
        phase_mlp_h1(0)
        phase_pv_half(1, 0)
        phase_weighted_half(1, 0)
        phase_pv_half(1, 1)
        phase_weighted_half(1, 1)
        phase_mlp_out(0)
        phase_mlp_h1(1)
        phase_mlp_out(1)


# ---- host-side shard packing ----

def _tile_rows(a):
    """[T*128, N] -> [128, T*N]: partition-tiled T-layout, contiguous DMA."""
    t = a.shape[0] // 128
    return a.reshape(t, 128, a.shape[1]).transpose(1, 0, 2).reshape(128, -1)


def _tile_weight(w):
    """W^T [768h, 768o] -> [128, (ot, ht, 128)]: o-major packed lhsT tiles."""
    x = w.reshape(HT, 128, HT, 128)          # [ht, p, ot, o128]
    return x.transpose(1, 2, 0, 3).reshape(128, -1)


def _tile_rows_blocked(a, qb):
    """[768, NB*qb] -> [128, NB*(6*qb)]: per-block ht-major packing."""
    nb = a.shape[1] // qb
    x = a.reshape(HT, 128, nb, qb).transpose(1, 2, 0, 3)
    return x.reshape(128, -1)


def shard_inputs(query, key, value, Wq, bq, Wk, bk, Wo1, bo1, Wo2, bo2):
    """Full inputs -> per-core in_maps (host packing, fp8 cast, scale folds)."""
    scale = np.float32(1.0 / np.sqrt(np.float32(H)))

    def c8(x):
        return np.ascontiguousarray(
            np.clip(np.asarray(x, np.float32), -240, 240).astype(NP_FP8))

    def cb(x):
        return np.ascontiguousarray(np.asarray(x, np.float32).astype(NP_BF16))

    def cf(x):
        return np.ascontiguousarray(x.astype(np.float32))

    bo1p = bo1 - Wo1 @ bo2           # corrects for the +bo2 folded into w'
    shared = {
        "wqT": c8(_tile_weight(Wq.T * (scale * WQ_SCALE))),
        "wkT": c8(_tile_weight(Wk.T * WK_SCALE)),
        "wo1T": c8(_tile_weight(Wo1.T * WO1_SCALE)),
        "wo2T": c8(_tile_weight(Wo2.T * WO2_SCALE)),
        "biases": cf(np.concatenate([
            (bq * scale).reshape(HT, 128).T, bk.reshape(HT, 128).T,
            np.asarray(bo1p).reshape(HT, 128).T], axis=1)),
    }
    in_maps = []
    for core in range(N_CORES):
        b, half = divmod(core, 2)
        r0 = half * QCHUNK
        vTb = np.asarray(value[b]).T + np.asarray(bo2)[:, None]
        in_maps.append({
            "qT": c8(_tile_rows_blocked(query[b].T[:, r0: r0 + QCHUNK], QB)),
            "kT": c8(_tile_rows_blocked(key[b].T, QB)),
            "v": c8(_tile_rows(np.asarray(value[b]))),
            "vTb": cb(_tile_rows_blocked(vTb[:, r0: r0 + QCHUNK], QB)),
            **shared,
        })
    return in_maps


def gather_outputs(results):
    """Per-core outT [128, NQB*HT*QB] bf16 -> full [B, S, H] fp32."""
    out = np.empty((B, S, H), dtype=np.float32)
    for core in range(N_CORES):
        b, half = divmod(core, 2)
        r0 = half * QCHUNK
        buf = results[core]["outT"].reshape(128, NQB, HT, QB)
        # out[q0+qb*QB+n, ot*128+p] = buf[p, qb, ot, n]
        out[b, r0: r0 + QCHUNK] = (
            buf.transpose(1, 3, 2, 0).reshape(QCHUNK, H).astype(np.float32)
        )
    return out


def run(inputs, trace=False):
    nc = build_kernel()
    in_maps = shard_inputs(**{k: np.asarray(v) for k, v in inputs.items()})
    res = run_bass_kernel_spmd(nc, in_maps, list(range(N_CORES)), trace=trace)
    return gather_outputs(res.results), res


def _split_multi_waits(nc):
    """Workaround for this container's walrus rejecting instructions that
    carry more than one semaphore wait ("Too many sync wait commands"):
    hoist N-1 waits onto fresh single-wait same-engine InstNoOp instructions
    inserted immediately before the instruction. Engine streams execute the
    block's per-engine subsequence in order, so blocking on the nops first is
    semantically identical to one multi-wait instruction."""
    for f in nc.m.functions:
        for bb in f.blocks:
            insts = list(bb.instructions)
            out = []
            changed = False
            for inst in insts:
                si = inst.sync_info
                waits = list(si.on_wait) if si is not None and si.on_wait else []
                if len(waits) > 1:
                    changed = True
                    for w in waits[:-1]:
                        nop = mybir.InstNoOp(
                            name=nc.get_next_instruction_name(), ins=[], outs=[]
                        )
                        nop.engine = inst.engine
                        nop.sync_info = mybir.SyncInfo(on_wait=[w], on_update=[])
                        out.append(nop)
                    si.on_wait = waits[-1:]
                    inst.sync_info = si
                out.append(inst)
            if changed:
                bb.instructions = out


def kernel(**inputs):
    """Entry point: full (unsharded) numpy inputs -> full [B, S, H] output."""
    out, _ = run(inputs, trace=False)
    return out


# revision 22
# speedup vs baseline: 1.1722x; 1.1722x over previous
"""Distributed single-head attention + MLP block for 8 TRN2 NeuronCores.

Reference computation (per batch b):
  Q = query @ Wq^T + bq ; K = key @ Wk^T + bk
  scores = Q @ K^T / sqrt(H) ; attn = softmax(scores)
  weighted = attn @ value + value
  h1 = relu(weighted @ Wo1^T + bo1)
  out = h1 @ Wo2^T + bo2 + weighted

Sharding: B=4 batches x 2 query-row halves = 8 shards. Each core gets its
1024 query rows plus the full 2048 keys/values of its batch; attention is
dense (non-causal) so no inter-core communication is needed.

Layout strategy: everything on-device lives feature-on-partitions
("T-layout", X^T[f, tok]) so all matmul contractions line up with zero
on-device transposes, and the host pre-packs every shard into the exact
[128, free] SBUF tiling the kernel consumes. All five GEMMs run in fp8
DoubleRow (2 contraction rows/cycle); fp8 weight tensors are scaled up
x64/x256 on the host to escape the e4m3 subnormal range and un-scaled for
free via the ScalarE activation's scale argument. The softmax needs no
max-subtraction: scores have std ~1/3 by construction.

Residual/bias algebra: the host ships vTb = value^T + bo2, so the kernel's
"weighted + bo2" residual costs nothing; feeding the MLP with w' = w + bo2
is corrected by bo1' = bo1 - Wo1 @ bo2 (exact), which lets the second MLP
GEMM skip ScalarE entirely (one DVE add straight out of PSUM).

PSUM is laid out as 2x rotating [128,1024] bank-pairs (scores/projection/
MLP accumulators, evacuated by ONE wide ScalarE activation each) + 3
single banks for the PV accumulation (processed in two ht-halves) + 1
norm/warmup bank. The softmax denominator is a 4-op contiguous DVE fold
tree over the [128, 16*512] exp tile, then the usual ones-matmul rowsum +
reciprocal + PE-broadcast. A dozen throwaway matmuls run during the
initial DMA wait to trip the PE HAM clock-gate to 2.4 GHz before the real
GEMM stream starts.
"""

import contextlib

import numpy as np
import ml_dtypes

import concourse.bass as bass
import concourse.mybir as mybir
import concourse.tile as tile
from concourse.bass_utils import run_bass_kernel_spmd

dt = mybir.dt
AF = mybir.ActivationFunctionType

H = 768          # model dim
B = 4            # batch
S = 2048         # sequence length
N_CORES = 8
QCHUNK = S * B // N_CORES        # 1024 query rows per core
HT = H // 128                    # 6 feature partition-tiles
KTILES = S // 128                # 16 key partition-tiles
QB = 512                         # q-block width (= PSUM bank, fp32)
NQB = QCHUNK // QB               # 2 q-blocks per core

FP8 = dt.float8e4
NP_FP8 = dt.np(FP8)
BF16 = dt.bfloat16
NP_BF16 = ml_dtypes.bfloat16
PMODE = mybir.MatmulPerfMode.DoubleRow

WQ_SCALE = 256.0                 # host premultiplier on Wq (incl 1/sqrt(H))
WK_SCALE = 64.0                  # host premultiplier on Wk
WO1_SCALE = 64.0                 # host premultiplier on Wo1
WO2_SCALE = 64.0                 # host premultiplier on Wo2


def build_kernel():
    nc = bass.Bass()

    qT_ext = nc.declare_dram_parameter("qT", [128, HT * QCHUNK], FP8, isOutput=False)
    kT_ext = nc.declare_dram_parameter("kT", [128, HT * S], FP8, isOutput=False)
    v_ext = nc.declare_dram_parameter("v", [128, KTILES * H], FP8, isOutput=False)
    vTb_ext = nc.declare_dram_parameter("vTb", [128, HT * QCHUNK], BF16,
                                        isOutput=False)
    w_ext = {
        name: nc.declare_dram_parameter(name, [128, HT * H], FP8, isOutput=False)
        for name in ("wqT", "wkT", "wo1T", "wo2T")
    }
    b_ext = nc.declare_dram_parameter("biases", [128, 3 * HT], dt.float32,
                                      isOutput=False)
    outT_ext = nc.declare_dram_parameter(
        "outT", [128, HT * QCHUNK], BF16, isOutput=True
    )

    with tile.TileContext(nc) as tc, nc.allow_low_precision(
        reason="fp8 matmul path is intentional; rel-err budget is 2e-2"
    ):
        _body(nc, tc, qT_ext, kT_ext, v_ext, vTb_ext, w_ext, b_ext, outT_ext)

    _split_multi_waits(nc)
    return nc


def _body(nc, tc, qT_ext, kT_ext, v_ext, vTb_ext, w_ext, b_ext, outT_ext):
    with contextlib.ExitStack() as ctx:
        const_pool = ctx.enter_context(tc.tile_pool(name="const", bufs=1))
        w_pool = ctx.enter_context(tc.tile_pool(name="w", bufs=1))
        act_pool = ctx.enter_context(tc.tile_pool(name="act", bufs=1))
        st_pool = ctx.enter_context(tc.tile_pool(name="st", bufs=1))
        out_pool = ctx.enter_context(tc.tile_pool(name="outs", bufs=3))
        # PSUM: 2 x [128,1024] rotating bank-pairs + 3 PV banks + 1 norm bank.
        ps_pair = ctx.enter_context(tc.tile_pool(name="ps_pair", bufs=2,
                                                 space="PSUM"))
        ps_one = ctx.enter_context(tc.tile_pool(name="ps_one", bufs=1,
                                                space="PSUM"))

        # ---- constants + PE warm-up (no DMA dependency: runs during the
        # input DMA head and trips the HAM clock gate to full rate) ----
        wu = const_pool.tile([128, 256], BF16, tag="warmup")
        nc.vector.memset(wu[:], 0.002)
        ones_f32 = const_pool.tile([128, 128], dt.float32, tag="ones_f32")
        nc.vector.memset(ones_f32[:], 1.0)
        ones_row = const_pool.tile([1, 128], dt.float32r, tag="ones_row")
        nc.vector.tensor_copy(ones_row[:], ones_f32[0:1, :])
        ones_col = const_pool.tile([128, 1], dt.float32r, tag="ones_col")
        nc.vector.tensor_copy(ones_col[:], ones_f32[:, 0:1])
        # fp8 ones pair for the qb1 denominator matmul: DR lhsT needs the
        # k-tile step to be a multiple of 16 bytes, so cols 0 and 16 of a
        # 32-wide tile are the two "rows" the AP actually reads.
        ones8 = const_pool.tile([128, 32], FP8, tag="ones8")
        nc.vector.memset(ones8[:], 1.0)
        ones8v = ones8[:].rearrange("p (t m) -> p t m", t=2)[:, :, 0:1]

        # ---- input DMAs in first-use order. The first-matmul chain (kT
        # block A, wkT chunk 0, biases, rest of wkT, kT block B) rides the
        # sync queue; ScalarE's queue opens with a dummy Ln activation that
        # pre-pays the ~2.7us ACT table load while those DMAs fly (Ln selects
        # natural_log_exp_and_others, which also holds Exp/Relu/Identity/
        # Copy, so this is the only table load in the kernel). ----
        actwarm = const_pool.tile([1, 2], dt.float32, tag="actwarm")
        nc.scalar.activation(actwarm[:], ones_f32[0:1, 0:2], AF.Ln)

        kT_in = act_pool.tile([128, HT * S], FP8, tag="kT_in")
        nc.sync.dma_start(kT_in[:, : HT * QB * 2], kT_ext[:, : HT * QB * 2])
        w_sb = {
            name: w_pool.tile([128, HT * H], FP8, tag=name, name=f"w_{name}")
            for name in ("wqT", "wkT", "wo1T", "wo2T")
        }
        nc.sync.dma_start(w_sb["wkT"][:, : HT * 128],
                          w_ext["wkT"][:, : HT * 128])
        nc.sync.dma_start(w_sb["wkT"][:, HT * 128:], w_ext["wkT"][:, HT * 128:])
        bias_sb = const_pool.tile([128, 3 * HT], dt.float32, tag="biases")
        nc.sync.dma_start(bias_sb[:], b_ext[:])
        biases = {name: bias_sb[:, i * HT:(i + 1) * HT]
                  for i, name in enumerate(("bq", "bk", "bo1"))}
        nc.sync.dma_start(kT_in[:, HT * QB * 2:], kT_ext[:, HT * QB * 2:])
        nc.scalar.dma_start(w_sb["wqT"][:], w_ext["wqT"][:])
        qT_in = act_pool.tile([128, HT * QCHUNK], FP8, tag="qT_in")
        nc.scalar.dma_start(qT_in[:], qT_ext[:])
        v_sb = act_pool.tile([128, KTILES * H], FP8, tag="v_in")
        nc.sync.dma_start(v_sb[:], v_ext[:])
        nc.sync.dma_start(w_sb["wo1T"][:], w_ext["wo1T"][:])
        nc.sync.dma_start(w_sb["wo2T"][:], w_ext["wo2T"][:])
        vTb_in = act_pool.tile([128, HT * QCHUNK], BF16, tag="vTb_in")
        nc.sync.dma_start(vTb_in[:], vTb_ext[:])

        # warm-up matmuls (~2.6us of junk PE work into the norm bank)
        for i in range(12):
            ps_wu = ps_one.tile([128, QB], dt.float32, tag="norm",
                                name=f"wu{i}")
            nc.tensor.matmul(ps_wu[:, :256], wu[:, :128], wu[:],
                             start=True, stop=True)

        def w3(name):
            return w_sb[name][:].rearrange("p (o t m) -> p (o t) m", o=HT, t=HT)

        # ---- K/Q projections: jo-contraction into [128,1024] bank-pairs,
        # one wide ScalarE activation per (ot, block-pair) ----
        def project(wname, bias, scale, x_in, nblocks, out_sb, out_col, tag):
            wv = w3(wname)
            xv = x_in[:].rearrange("p (n t q) -> p n t q", n=nblocks, t=HT)
            for nb2 in range(nblocks // 2):
                for ot in range(HT):
                    pair = ps_pair.tile([128, 2 * QB], dt.float32, tag="pair",
                                        name=f"ps_{tag}_{nb2}_{ot}")
                    for jo in range(HT // 2):
                        for h in range(2):
                            nc.tensor.matmul(
                                pair[:, h * QB:(h + 1) * QB],
                                wv[:, ot * HT + 2 * jo: ot * HT + 2 * jo + 2, :],
                                xv[:, 2 * nb2 + h, 2 * jo: 2 * jo + 2, :],
                                start=(jo == 0),
                                stop=(jo == HT // 2 - 1),
                                perf_mode=PMODE,
                            )
                    c0 = out_col(ot, nb2)
                    nc.scalar.activation(
                        out_sb[:, c0: c0 + 2 * QB], pair[:], AF.Identity,
                        bias=bias[:, ot: ot + 1], scale=scale,
                    )

        KT = act_pool.tile([128, HT * S], FP8, tag="KT", name="KT_full")
        project("wkT", biases["bk"], 1.0 / WK_SCALE, kT_in, 4, KT,
                lambda ot, nb2: ot * S + nb2 * 2 * QB, "KT")
        QT = act_pool.tile([128, HT * QCHUNK], FP8, tag="QT", name="proj_QT")
        project("wqT", biases["bq"], 1.0 / WQ_SCALE, qT_in, 2, QT,
                lambda ot, nb2: ot * QCHUNK + nb2 * 2 * QB, "QT")
        KT3 = KT[:].rearrange("p (t k) -> p t k", t=HT)
        QT3 = QT[:].rearrange("p (t q) -> p t q", t=HT)

        def vpair(jk, ht):
            """lhsT [128, 2, 128]: k-tile pair (2jk, 2jk+1), h-tile ht."""
            return (v_sb[:].rearrange("p (t h) -> p t h", t=KTILES)
                    [:, 2 * jk: 2 * jk + 2, ht * 128:(ht + 1) * 128])

        # ---- attention + MLP, software-pipelined across q-blocks ----
        state = {}

        def phase_scores(qb):
            """scores^T + exp, two k-tiles per PSUM bank-pair / ACTIVATE;
            the denominator accumulates on the (otherwise idle) DVE as
            incremental [128,1024] adds trailing the exps."""
            q0 = qb * QB
            expT = st_pool.tile([128, KTILES * QB], FP8, tag=f"expT{qb}",
                                name=f"expT{qb}")
            acc = st_pool.tile([128, 2 * QB], BF16, tag=f"acc{qb}",
                               name=f"acc{qb}")
            state[qb] = {"expT": expT}
            for p8 in range(KTILES // 2):
                pair = ps_pair.tile([128, 2 * QB], dt.float32, tag="pair",
                                    name=f"ps_s_{qb}_{p8}")
                for half in range(2):
                    kt = 2 * p8 + half
                    for jo in range(HT // 2):
                        nc.tensor.matmul(
                            pair[:, half * QB:(half + 1) * QB],
                            KT3[:, 2 * jo: 2 * jo + 2, kt * 128:(kt + 1) * 128],
                            QT3[:, 2 * jo: 2 * jo + 2, q0: q0 + QB],
                            start=(jo == 0),
                            stop=(jo == HT // 2 - 1),
                            perf_mode=PMODE,
                        )
                sl = expT[:, p8 * 2 * QB:(p8 + 1) * 2 * QB]
                nc.scalar.activation(sl, pair[:], AF.Exp)
                if p8 == 0:
                    nc.vector.tensor_copy(acc[:], sl)
                else:
                    nc.vector.tensor_add(acc[:], acc[:], sl)
            sum_part = st_pool.tile([128, QB], dt.float32r, tag=f"sump{qb}",
                                    name=f"sum_part{qb}")
            nc.vector.tensor_add(sum_part[:], acc[:, :QB], acc[:, QB:])
            state[qb]["sum_part"] = sum_part

        def phase_norm_ln(qb):
            """Partition-reduce the rowsum with one ones-matmul, then
            1/rowsum = exp(-ln(rowsum)) on ScalarE: no DVE reciprocal."""
            st = state[qb]
            ps_den = ps_one.tile([128, QB], dt.float32, tag="norm",
                                 name=f"ps_den{qb}")
            nc.tensor.matmul(ps_den[0:1, :], ones_col[:], st["sum_part"][:],
                             start=True, stop=True)
            logsum = st_pool.tile([1, QB], dt.float32r, tag="logsum",
                                  name=f"logsum{qb}")
            nc.scalar.activation(logsum[:], ps_den[0:1, :], AF.Ln)
            st["logsum"] = logsum

        def phase_norm_bcast(qb):
            st = state[qb]
            ps_b = ps_one.tile([128, QB], dt.float32, tag="norm",
                               name=f"ps_b{qb}")
            nc.tensor.matmul(ps_b[:], ones_row[:], st["logsum"][:],
                             start=True, stop=True)
            bcast = st_pool.tile([128, QB], dt.float32, tag="bcast",
                                 name=f"bcast{qb}")
            nc.scalar.activation(bcast[:], ps_b[:], AF.Exp, scale=-1.0)
            st["bcast"] = bcast

        def phase_pv_half(qb, half):
            """PV for 3 h-tiles over all 16 k-tiles; norm chain of this
            q-block interleaves under half 0."""
            st = state[qb]
            rhs8 = st["expT"][:].rearrange("p (j t q) -> p j t q",
                                           j=KTILES // 2, t=2)
            ps_w = [ps_one.tile([128, QB], dt.float32, tag=f"pvw{i}",
                                name=f"pvw{i}_{qb}_{half}")
                    for i in range(3)]
            for jk in range(KTILES // 2):
                if half == 0 and jk == 1:
                    phase_norm_ln(qb)
                if half == 0 and jk == 3:
                    phase_norm_bcast(qb)
                for i in range(3):
                    nc.tensor.matmul(
                        ps_w[i][:],
                        vpair(jk, 3 * half + i),
                        rhs8[:, jk],
                        start=(jk == 0),
                        stop=(jk == KTILES // 2 - 1),
                        perf_mode=PMODE,
                    )
            st[f"ps_w{half}"] = ps_w

        def phase_weighted_half(qb, half):
            """w = PV/rowsum + (value^T + bo2); bf16 residual + fp8 GEMM copy."""
            st = state[qb]
            ps_w = st[f"ps_w{half}"]
            if "wr" not in st:
                st["wr"] = st_pool.tile([128, HT * QB], BF16, tag=f"wr{qb}",
                                        name=f"wr{qb}")
                st["w8"] = st_pool.tile([128, HT * QB], FP8, tag=f"w8_{qb}",
                                        name=f"w8_{qb}")
            wr, w8 = st["wr"], st["w8"]
            for i in range(3):
                ht = 3 * half + i
                c0 = ht * QB
                tmp = out_pool.tile([128, QB], dt.float32, tag="wtmp",
                                    name=f"wtmp_{qb}_{ht}")
                nc.vector.tensor_mul(tmp[:], ps_w[i][:], st["bcast"][:])
                nc.vector.tensor_add(
                    wr[:, c0: c0 + QB], tmp[:],
                    vTb_in[:, qb * HT * QB + c0: qb * HT * QB + c0 + QB],
                )
                nc.scalar.copy(w8[:, c0: c0 + QB], wr[:, c0: c0 + QB])

        def phase_mlp_h1(qb):
            st = state[qb]
            w8v = st["w8"][:].rearrange("p (t q) -> p t q", t=HT)
            wv = w3("wo1T")
            h1 = st_pool.tile([128, HT * QB], FP8, tag=f"h1_{qb}",
                              name=f"h1T{qb}")
            for otp in range(HT // 2):
                pair = ps_pair.tile([128, 2 * QB], dt.float32, tag="pair",
                                    name=f"ps_h1_{qb}_{otp}")
                for h in range(2):
                    ot = 2 * otp + h
                    for jo in range(HT // 2):
                        nc.tensor.matmul(
                            pair[:, h * QB:(h + 1) * QB],
                            wv[:, ot * HT + 2 * jo: ot * HT + 2 * jo + 2, :],
                            w8v[:, 2 * jo: 2 * jo + 2, :],
                            start=(jo == 0),
                            stop=(jo == HT // 2 - 1),
                            perf_mode=PMODE,
                        )
                for h in range(2):
                    ot = 2 * otp + h
                    nc.scalar.activation(
                        h1[:, ot * QB:(ot + 1) * QB],
                        pair[:, h * QB:(h + 1) * QB],
                        AF.Relu, bias=biases["bo1"][:, ot: ot + 1],
                        scale=1.0 / WO1_SCALE,
                    )
            st["h1"] = h1

        def phase_mlp_out(qb):
            """out = h1 @ Wo2^T + (w + bo2): PSUM + residual in one DVE add,
            straight to bf16 DMA staging."""
            st = state[qb]
            h1v = st["h1"][:].rearrange("p (t q) -> p t q", t=HT)
            wv = w3("wo2T")
            for otp in range(HT // 2):
                pair = ps_pair.tile([128, 2 * QB], dt.float32, tag="pair",
                                    name=f"ps_o_{qb}_{otp}")
                for h in range(2):
                    ot = 2 * otp + h
                    for jo in range(HT // 2):
                        nc.tensor.matmul(
                            pair[:, h * QB:(h + 1) * QB],
                            wv[:, ot * HT + 2 * jo: ot * HT + 2 * jo + 2, :],
                            h1v[:, 2 * jo: 2 * jo + 2, :],
                            start=(jo == 0),
                            stop=(jo == HT // 2 - 1),
                            perf_mode=PMODE,
                        )
                o_mid = out_pool.tile([128, 2 * QB], BF16, tag="o_mid",
                                      name=f"omid_{qb}_{otp}")
                o_sb = out_pool.tile([128, 2 * QB], BF16, tag="outT_blk",
                                     name=f"outT_{qb}_{otp}")
                c0 = (qb * HT + otp * 2) * QB
                # the very last block streams out in 512-wide chunks so the
                # act -> add -> DMA tail pipeline overlaps
                nchunk = 2 if (qb, otp) == (1, HT // 2 - 1) else 1
                for ch in range(nchunk):
                    w = 2 * QB // nchunk
                    sl = slice(ch * w, (ch + 1) * w)
                    nc.scalar.activation(o_mid[:, sl], pair[:, sl], AF.Identity,
                                         scale=1.0 / WO2_SCALE)
                    nc.vector.tensor_add(
                        o_sb[:, sl], o_mid[:, sl],
                        st["wr"][:, otp * 2 * QB + ch * w:
                                 otp * 2 * QB + (ch + 1) * w],
                    )
                    nc.sync.dma_start(
                        outT_ext[:, c0 + ch * w: c0 + (ch + 1) * w],
                        o_sb[:, sl],
                    )

        # software pipeline: DVE/ScalarE chains (norm, weighted, h1-acts) are
        # always covered by an independent PE phase emitted around them
        phase_scores(0)
        phase_pv_half(0, 0)
        phase_weighted_half(0, 0)
        phase_pv_half(0, 1)
        phase_weighted_half(0, 1)
        phase_scores(1)
        phase_mlp_h1(0)
        phase_pv_half(1, 0)
        phase_weighted_half(1, 0)
        phase_pv_half(1, 1)
        phase_weighted_half(1, 1)
        phase_mlp_out(0)
        phase_mlp_h1(1)
        phase_mlp_out(1)


# ---- host-side shard packing ----

def _tile_rows(a):
    """[T*128, N] -> [128, T*N]: partition-tiled T-layout, contiguous DMA."""
    t = a.shape[0] // 128
    return a.reshape(t, 128, a.shape[1]).transpose(1, 0, 2).reshape(128, -1)


def _tile_weight(w):
    """W^T [768h, 768o] -> [128, (ot, ht, 128)]: o-major packed lhsT tiles."""
    x = w.reshape(HT, 128, HT, 128)          # [ht, p, ot, o128]
    return x.transpose(1, 2, 0, 3).reshape(128, -1)


def _tile_rows_blocked(a, qb):
    """[768, NB*qb] -> [128, NB*(6*qb)]: per-block ht-major packing."""
    nb = a.shape[1] // qb
    x = a.reshape(HT, 128, nb, qb).transpose(1, 2, 0, 3)
    return x.reshape(128, -1)


def shard_inputs(query, key, value, Wq, bq, Wk, bk, Wo1, bo1, Wo2, bo2):
    """Full inputs -> per-core in_maps (host packing, fp8 cast, scale folds)."""
    scale = np.float32(1.0 / np.sqrt(np.float32(H)))

    def c8(x):
        return np.ascontiguousarray(
            np.clip(np.asarray(x, np.float32), -240, 240).astype(NP_FP8))

    def cb(x):
        return np.ascontiguousarray(np.asarray(x, np.float32).astype(NP_BF16))

    def cf(x):
        return np.ascontiguousarray(x.astype(np.float32))

    bo1p = bo1 - Wo1 @ bo2           # corrects for the +bo2 folded into w'
    shared = {
        "wqT": c8(_tile_weight(Wq.T * (scale * WQ_SCALE))),
        "wkT": c8(_tile_weight(Wk.T * WK_SCALE)),
        "wo1T": c8(_tile_weight(Wo1.T * WO1_SCALE)),
        "wo2T": c8(_tile_weight(Wo2.T * WO2_SCALE)),
        "biases": cf(np.concatenate([
            (bq * scale).reshape(HT, 128).T, bk.reshape(HT, 128).T,
            np.asarray(bo1p).reshape(HT, 128).T], axis=1)),
    }
    in_maps = []
    for core in range(N_CORES):
        b, half = divmod(core, 2)
        r0 = half * QCHUNK
        vTb = np.asarray(value[b]).T + np.asarray(bo2)[:, None]
        in_maps.append({
            "qT": c8(_tile_rows_blocked(query[b].T[:, r0: r0 + QCHUNK], QB)),
            "kT": c8(_tile_rows_blocked(key[b].T, QB)),
            "v": c8(_tile_rows(np.asarray(value[b]))),
            "vTb": cb(_tile_rows_blocked(vTb[:, r0: r0 + QCHUNK], QB)),
            **shared,
        })
    return in_maps


def gather_outputs(results):
    """Per-core outT [128, NQB*HT*QB] bf16 -> full [B, S, H] fp32."""
    out = np.empty((B, S, H), dtype=np.float32)
    for core in range(N_CORES):
        b, half = divmod(core, 2)
        r0 = half * QCHUNK
        buf = results[core]["outT"].reshape(128, NQB, HT, QB)
        # out[q0+qb*QB+n, ot*128+p] = buf[p, qb, ot, n]
        out[b, r0: r0 + QCHUNK] = (
            buf.transpose(1, 3, 2, 0).reshape(QCHUNK, H).astype(np.float32)
        )
    return out


def run(inputs, trace=False):
    nc = build_kernel()
    in_maps = shard_inputs(**{k: np.asarray(v) for k, v in inputs.items()})
    res = run_bass_kernel_spmd(nc, in_maps, list(range(N_CORES)), trace=trace)
    return gather_outputs(res.results), res


def _split_multi_waits(nc):
    """Workaround for this container's walrus rejecting instructions that
    carry more than one semaphore wait ("Too many sync wait commands"):
    hoist N-1 waits onto fresh single-wait same-engine InstNoOp instructions
    inserted immediately before the instruction. Engine streams execute the
    block's per-engine subsequence in order, so blocking on the nops first is
    semantically identical to one multi-wait instruction."""
    for f in nc.m.functions:
        for bb in f.blocks:
            insts = list(bb.instructions)
            out = []
            changed = False
            for inst in insts:
                si = inst.sync_info
                waits = list(si.on_wait) if si is not None and si.on_wait else []
                if len(waits) > 1:
                    changed = True
                    for w in waits[:-1]:
                        nop = mybir.InstNoOp(
                            name=nc.get_next_instruction_name(), ins=[], outs=[]
                        )
                        nop.engine = inst.engine
                        nop.sync_info = mybir.SyncInfo(on_wait=[w], on_update=[])
                        out.append(nop)
                    si.on_wait = waits[-1:]
                    inst.sync_info = si
                out.append(inst)
            if changed:
                bb.instructions = out


def kernel(**inputs):
    """Entry point: full (unsharded) numpy inputs -> full [B, S, H] output."""
    out, _ = run(inputs, trace=False)
    return out


# revision 27
# speedup vs baseline: 1.2221x; 1.0426x over previous
"""Distributed single-head attention + MLP block for 8 TRN2 NeuronCores.

Reference computation (per batch b):
  Q = query @ Wq^T + bq ; K = key @ Wk^T + bk
  scores = Q @ K^T / sqrt(H) ; attn = softmax(scores)
  weighted = attn @ value + value
  h1 = relu(weighted @ Wo1^T + bo1)
  out = h1 @ Wo2^T + bo2 + weighted

Sharding: B=4 batches x 2 query-row halves = 8 shards. Each core gets its
1024 query rows plus the full 2048 keys/values of its batch; attention is
dense (non-causal) so no inter-core communication is needed.

Layout strategy: everything on-device lives feature-on-partitions
("T-layout", X^T[f, tok]) so all matmul contractions line up with zero
on-device transposes, and the host pre-packs every shard into the exact
[128, free] SBUF tiling the kernel consumes. All five GEMMs run in fp8
DoubleRow (2 contraction rows/cycle); fp8 weight tensors are scaled up
x64/x256 on the host to escape the e4m3 subnormal range and un-scaled for
free via the ScalarE activation's scale argument. The softmax needs no
max-subtraction: scores have std ~1/3 by construction.

Residual/bias algebra: the host ships vTb = value^T + bo2, so the kernel's
"weighted + bo2" residual costs nothing; feeding the MLP with w' = w + bo2
is corrected by bo1' = bo1 - Wo1 @ bo2 (exact), which lets the second MLP
GEMM skip ScalarE entirely (one DVE add straight out of PSUM).

PSUM is laid out as 2x rotating [128,1024] bank-pairs (scores/projection/
MLP accumulators, evacuated by ONE wide ScalarE activation each) + 3
single banks for the PV accumulation (processed in two ht-halves) + 1
norm/warmup bank. The softmax denominator is a 4-op contiguous DVE fold
tree over the [128, 16*512] exp tile, then the usual ones-matmul rowsum +
reciprocal + PE-broadcast. A dozen throwaway matmuls run during the
initial DMA wait to trip the PE HAM clock-gate to 2.4 GHz before the real
GEMM stream starts.
"""

import contextlib

import numpy as np
import ml_dtypes

import concourse.bass as bass
import concourse.mybir as mybir
import concourse.tile as tile
from concourse.bass_utils import run_bass_kernel_spmd

dt = mybir.dt
AF = mybir.ActivationFunctionType

H = 768          # model dim
B = 4            # batch
S = 2048         # sequence length
N_CORES = 8
QCHUNK = S * B // N_CORES        # 1024 query rows per core
HT = H // 128                    # 6 feature partition-tiles
KTILES = S // 128                # 16 key partition-tiles
QB = 512                         # q-block width (= PSUM bank, fp32)
NQB = QCHUNK // QB               # 2 q-blocks per core

FP8 = dt.float8e4
NP_FP8 = dt.np(FP8)
BF16 = dt.bfloat16
NP_BF16 = ml_dtypes.bfloat16
PMODE = mybir.MatmulPerfMode.DoubleRow

WQ_SCALE = 256.0                 # host premultiplier on Wq (incl 1/sqrt(H))
WK_SCALE = 64.0                  # host premultiplier on Wk
WO1_SCALE = 64.0                 # host premultiplier on Wo1
WO2_SCALE = 64.0                 # host premultiplier on Wo2


def build_kernel():
    nc = bass.Bass()

    qT_ext = nc.declare_dram_parameter("qT", [128, HT * QCHUNK], FP8, isOutput=False)
    kT_ext = nc.declare_dram_parameter("kT", [128, HT * S], FP8, isOutput=False)
    v_ext = nc.declare_dram_parameter("v", [128, KTILES * H], FP8, isOutput=False)
    vTb_ext = nc.declare_dram_parameter("vTb", [128, HT * QCHUNK], BF16,
                                        isOutput=False)
    w_ext = {
        name: nc.declare_dram_parameter(name, [128, HT * H], FP8, isOutput=False)
        for name in ("wqT", "wkT", "wo1T", "wo2T")
    }
    b_ext = nc.declare_dram_parameter("biases", [128, 3 * HT], dt.float32,
                                      isOutput=False)
    outT_ext = nc.declare_dram_parameter(
        "outT", [128, HT * QCHUNK], BF16, isOutput=True
    )

    with tile.TileContext(nc) as tc, nc.allow_low_precision(
        reason="fp8 matmul path is intentional; rel-err budget is 2e-2"
    ):
        _body(nc, tc, qT_ext, kT_ext, v_ext, vTb_ext, w_ext, b_ext, outT_ext)

    _split_multi_waits(nc)
    return nc


def _body(nc, tc, qT_ext, kT_ext, v_ext, vTb_ext, w_ext, b_ext, outT_ext):
    with contextlib.ExitStack() as ctx:
        const_pool = ctx.enter_context(tc.tile_pool(name="const", bufs=1))
        w_pool = ctx.enter_context(tc.tile_pool(name="w", bufs=1))
        act_pool = ctx.enter_context(tc.tile_pool(name="act", bufs=1))
        st_pool = ctx.enter_context(tc.tile_pool(name="st", bufs=1))
        out_pool = ctx.enter_context(tc.tile_pool(name="outs", bufs=3))
        # PSUM: 2 x [128,1024] rotating bank-pairs + 3 PV banks + 1 norm bank.
        ps_pair = ctx.enter_context(tc.tile_pool(name="ps_pair", bufs=2,
                                                 space="PSUM"))
        ps_one = ctx.enter_context(tc.tile_pool(name="ps_one", bufs=1,
                                                space="PSUM"))

        # ---- constants + PE warm-up (no DMA dependency: runs during the
        # input DMA head and trips the HAM clock gate to full rate) ----
        wu = const_pool.tile([128, 256], BF16, tag="warmup")
        nc.vector.memset(wu[:], 0.002)
        ones_f32 = const_pool.tile([128, 128], dt.float32, tag="ones_f32")
        nc.vector.memset(ones_f32[:], 1.0)
        ones_row = const_pool.tile([1, 128], dt.float32r, tag="ones_row")
        nc.vector.tensor_copy(ones_row[:], ones_f32[0:1, :])
        ones_col = const_pool.tile([128, 1], dt.float32r, tag="ones_col")
        nc.vector.tensor_copy(ones_col[:], ones_f32[:, 0:1])
        # fp8 ones pair for the qb1 denominator matmul: DR lhsT needs the
        # k-tile step to be a multiple of 16 bytes, so cols 0 and 16 of a
        # 32-wide tile are the two "rows" the AP actually reads.
        ones8 = const_pool.tile([128, 32], FP8, tag="ones8")
        nc.vector.memset(ones8[:], 1.0)
        ones8v = ones8[:].rearrange("p (t m) -> p t m", t=2)[:, :, 0:1]

        # ---- input DMAs in first-use order. The first-matmul chain (kT
        # block A, wkT chunk 0, biases, rest of wkT, kT block B) rides the
        # sync queue; ScalarE's queue opens with a dummy Ln activation that
        # pre-pays the ~2.7us ACT table load while those DMAs fly (Ln selects
        # natural_log_exp_and_others, which also holds Exp/Relu/Identity/
        # Copy, so this is the only table load in the kernel). ----
        actwarm = const_pool.tile([1, 2], dt.float32, tag="actwarm")
        nc.scalar.activation(actwarm[:], ones_f32[0:1, 0:2], AF.Ln)

        kT_in = act_pool.tile([128, HT * S], FP8, tag="kT_in")
        nc.sync.dma_start(kT_in[:, : HT * QB * 2], kT_ext[:, : HT * QB * 2])
        w_sb = {
            name: w_pool.tile([128, HT * H], FP8, tag=name, name=f"w_{name}")
            for name in ("wqT", "wkT", "wo1T", "wo2T")
        }
        nc.sync.dma_start(w_sb["wkT"][:, : HT * 128],
                          w_ext["wkT"][:, : HT * 128])
        nc.sync.dma_start(w_sb["wkT"][:, HT * 128:], w_ext["wkT"][:, HT * 128:])
        bias_sb = const_pool.tile([128, 3 * HT], dt.float32, tag="biases")
        nc.sync.dma_start(bias_sb[:], b_ext[:])
        biases = {name: bias_sb[:, i * HT:(i + 1) * HT]
                  for i, name in enumerate(("bq", "bk", "bo1"))}
        nc.sync.dma_start(kT_in[:, HT * QB * 2:], kT_ext[:, HT * QB * 2:])
        nc.scalar.dma_start(w_sb["wqT"][:], w_ext["wqT"][:])
        qT_in = act_pool.tile([128, HT * QCHUNK], FP8, tag="qT_in")
        nc.scalar.dma_start(qT_in[:], qT_ext[:])
        v_sb = act_pool.tile([128, KTILES * H], FP8, tag="v_in")
        nc.sync.dma_start(v_sb[:], v_ext[:])
        nc.sync.dma_start(w_sb["wo1T"][:], w_ext["wo1T"][:])
        nc.sync.dma_start(w_sb["wo2T"][:], w_ext["wo2T"][:])
        vTb_in = act_pool.tile([128, HT * QCHUNK], BF16, tag="vTb_in")
        nc.sync.dma_start(vTb_in[:], vTb_ext[:])

        # warm-up matmuls (~2.6us of junk PE work into the norm bank)
        for i in range(12):
            ps_wu = ps_one.tile([128, QB], dt.float32, tag="norm",
                                name=f"wu{i}")
            nc.tensor.matmul(ps_wu[:, :256], wu[:, :128], wu[:],
                             start=True, stop=True)

        def w3(name):
            return w_sb[name][:].rearrange("p (o t m) -> p (o t) m", o=HT, t=HT)

        # ---- K/Q projections: jo-contraction into [128,1024] bank-pairs,
        # one wide ScalarE activation per (ot, block-pair) ----
        def project(wname, bias, scale, x_in, nblocks, out_sb, out_col, tag,
                    head_split=False):
            wv = w3(wname)
            xv = x_in[:].rearrange("p (n t q) -> p n t q", n=nblocks, t=HT)
            head_banks = {(0, 0): "pvw0", (0, 1): "pvw1",
                          (1, 0): "pvw2", (1, 1): "norm"}
            for nb2 in range(nblocks // 2):
                for ot in range(HT):
                    # the very first evacuations ride 4 idle single banks so
                    # the act-latency ramp doesn't stall the bank-pair pool
                    split = head_split and nb2 == 0 and ot < 2
                    if split:
                        tiles = [ps_one.tile([128, QB], dt.float32,
                                             tag=head_banks[(ot, h)],
                                             name=f"ps_{tag}h_{ot}_{h}")
                                 for h in range(2)]
                        tgt = lambda h: tiles[h][:]
                    else:
                        pair = ps_pair.tile([128, 2 * QB], dt.float32,
                                            tag="pair",
                                            name=f"ps_{tag}_{nb2}_{ot}")
                        tgt = lambda h: pair[:, h * QB:(h + 1) * QB]
                    for jo in range(HT // 2):
                        for h in range(2):
                            nc.tensor.matmul(
                                tgt(h),
                                wv[:, ot * HT + 2 * jo: ot * HT + 2 * jo + 2, :],
                                xv[:, 2 * nb2 + h, 2 * jo: 2 * jo + 2, :],
                                start=(jo == 0),
                                stop=(jo == HT // 2 - 1),
                                perf_mode=PMODE,
                            )
                    c0 = out_col(ot, nb2)
                    if split:
                        for h in range(2):
                            nc.scalar.activation(
                                out_sb[:, c0 + h * QB: c0 + (h + 1) * QB],
                                tiles[h][:], AF.Identity,
                                bias=bias[:, ot: ot + 1], scale=scale,
                            )
                    else:
                        nc.scalar.activation(
                            out_sb[:, c0: c0 + 2 * QB], pair[:], AF.Identity,
                            bias=bias[:, ot: ot + 1], scale=scale,
                        )

        KT = act_pool.tile([128, HT * S], FP8, tag="KT", name="KT_full")
        project("wkT", biases["bk"], 1.0 / WK_SCALE, kT_in, 4, KT,
                lambda ot, nb2: ot * S + nb2 * 2 * QB, "KT", head_split=True)
        QT = act_pool.tile([128, HT * QCHUNK], FP8, tag="QT", name="proj_QT")
        project("wqT", biases["bq"], 1.0 / WQ_SCALE, qT_in, 2, QT,
                lambda ot, nb2: ot * QCHUNK + nb2 * 2 * QB, "QT")
        KT3 = KT[:].rearrange("p (t k) -> p t k", t=HT)
        QT3 = QT[:].rearrange("p (t q) -> p t q", t=HT)

        def vpair(jk, ht):
            """lhsT [128, 2, 128]: k-tile pair (2jk, 2jk+1), h-tile ht."""
            return (v_sb[:].rearrange("p (t h) -> p t h", t=KTILES)
                    [:, 2 * jk: 2 * jk + 2, ht * 128:(ht + 1) * 128])

        # ---- attention + MLP, software-pipelined across q-blocks ----
        state = {}

        def den_mm(qb, p8):
            """qb1 path: accumulate exp-pair p8 into the [1,512] rowsum via
            a ones matmul (contraction over 128 partitions x 2 k-tiles)."""
            st = state[qb]
            rhs8 = st["expT"][:].rearrange("p (j t q) -> p j t q",
                                           j=KTILES // 2, t=2)
            nc.tensor.matmul(
                st["ps_den"][0:1, :], ones8v, rhs8[:, p8],
                start=(p8 == 0), stop=(p8 == KTILES // 2 - 1),
                perf_mode=PMODE,
            )

        def phase_scores(qb):
            """scores^T + exp, two k-tiles per PSUM bank-pair / ACTIVATE.
            Denominator: qb0 accumulates on the (idle there) DVE as
            incremental [128,1024] adds; qb1's scores window already has
            weighted-qb0 DVE work, so its denominator rides the PE as tiny
            ones-matmuls trailing one pair behind the exps."""
            q0 = qb * QB
            expT = st_pool.tile([128, KTILES * QB], FP8, tag=f"expT{qb}",
                                name=f"expT{qb}")
            state[qb] = {"expT": expT}
            if qb == 0:
                acc = st_pool.tile([128, 2 * QB], BF16, tag="acc0")
            else:
                state[qb]["ps_den"] = ps_one.tile(
                    [128, QB], dt.float32, tag="norm", name="ps_den1")
            for p8 in range(KTILES // 2):
                pair = ps_pair.tile([128, 2 * QB], dt.float32, tag="pair",
                                    name=f"ps_s_{qb}_{p8}")
                for half in range(2):
                    kt = 2 * p8 + half
                    for jo in range(HT // 2):
                        nc.tensor.matmul(
                            pair[:, half * QB:(half + 1) * QB],
                            KT3[:, 2 * jo: 2 * jo + 2, kt * 128:(kt + 1) * 128],
                            QT3[:, 2 * jo: 2 * jo + 2, q0: q0 + QB],
                            start=(jo == 0),
                            stop=(jo == HT // 2 - 1),
                            perf_mode=PMODE,
                        )
                sl = expT[:, p8 * 2 * QB:(p8 + 1) * 2 * QB]
                nc.scalar.activation(sl, pair[:], AF.Exp)
                if qb == 0:
                    if p8 == 0:
                        nc.vector.tensor_copy(acc[:], sl)
                    else:
                        nc.vector.tensor_add(acc[:], acc[:], sl)
                elif p8 >= 1:
                    den_mm(qb, p8 - 1)
            if qb == 0:
                sum_part = st_pool.tile([128, QB], dt.float32r, tag="sump0")
                nc.vector.tensor_add(sum_part[:], acc[:, :QB], acc[:, QB:])
                state[qb]["sum_part"] = sum_part

        def phase_norm_ln(qb):
            """Partition-reduce the rowsum (qb0: one ones-matmul on the DVE
            partial; qb1: close the PE den group), then 1/rowsum =
            exp(-ln(rowsum)) on ScalarE: no DVE reciprocal."""
            st = state[qb]
            if qb == 0:
                ps_den = ps_one.tile([128, QB], dt.float32, tag="norm",
                                     name="ps_den0")
                nc.tensor.matmul(ps_den[0:1, :], ones_col[:],
                                 st["sum_part"][:], start=True, stop=True)
            else:
                den_mm(qb, KTILES // 2 - 1)
                ps_den = st["ps_den"]
            logsum = st_pool.tile([1, QB], dt.float32r, tag="logsum",
                                  name=f"logsum{qb}")
            nc.scalar.activation(logsum[:], ps_den[0:1, :], AF.Ln)
            st["logsum"] = logsum

        def phase_norm_bcast(qb):
            st = state[qb]
            ps_b = ps_one.tile([128, QB], dt.float32, tag="norm",
                               name=f"ps_b{qb}")
            nc.tensor.matmul(ps_b[:], ones_row[:], st["logsum"][:],
                             start=True, stop=True)
            bcast = st_pool.tile([128, QB], dt.float32, tag="bcast",
                                 name=f"bcast{qb}")
            nc.scalar.activation(bcast[:], ps_b[:], AF.Exp, scale=-1.0)
            st["bcast"] = bcast

        def phase_pv_half(qb, half):
            """PV for 3 h-tiles over all 16 k-tiles; norm chain of this
            q-block interleaves under half 0."""
            st = state[qb]
            rhs8 = st["expT"][:].rearrange("p (j t q) -> p j t q",
                                           j=KTILES // 2, t=2)
            ps_w = [ps_one.tile([128, QB], dt.float32, tag=f"pvw{i}",
                                name=f"pvw{i}_{qb}_{half}")
                    for i in range(3)]
            if half == 0:
                sched = [(jk, i) for jk in range(KTILES // 2) for i in range(3)]
            else:
                # skewed wavefront: bank i starts i waves late, so the first
                # matmul of each bank lands just after the previous half's
                # weighted-mul releases that bank (no lump wait on DVE)
                sched = [(w - i, i) for w in range(KTILES // 2 + 2)
                         for i in range(3) if 0 <= w - i < KTILES // 2]
            for n, (jk, i) in enumerate(sched):
                if half == 0 and (jk, i) == (1, 0):
                    phase_norm_ln(qb)
                if half == 0 and (jk, i) == (3, 0):
                    phase_norm_bcast(qb)
                nc.tensor.matmul(
                    ps_w[i][:],
                    vpair(jk, 3 * half + i),
                    rhs8[:, jk],
                    start=(jk == 0),
                    stop=(jk == KTILES // 2 - 1),
                    perf_mode=PMODE,
                )
            st[f"ps_w{half}"] = ps_w

        def phase_weighted_half(qb, half):
            """w = PV/rowsum + (value^T + bo2); bf16 residual + fp8 GEMM copy."""
            st = state[qb]
            ps_w = st[f"ps_w{half}"]
            if "wr" not in st:
                st["wr"] = st_pool.tile([128, HT * QB], BF16, tag=f"wr{qb}",
                                        name=f"wr{qb}")
                st["w8"] = st_pool.tile([128, HT * QB], FP8, tag=f"w8_{qb}",
                                        name=f"w8_{qb}")
            wr, w8 = st["wr"], st["w8"]
            for i in range(3):
                ht = 3 * half + i
                c0 = ht * QB
                tmp = out_pool.tile([128, QB], dt.float32, tag="wtmp",
                                    name=f"wtmp_{qb}_{ht}")
                nc.vector.tensor_mul(tmp[:], ps_w[i][:], st["bcast"][:])
                nc.vector.tensor_add(
                    wr[:, c0: c0 + QB], tmp[:],
                    vTb_in[:, qb * HT * QB + c0: qb * HT * QB + c0 + QB],
                )
                if (qb, half) == (0, 1):
                    # this cast's consumers sit behind scores1's exps on
                    # ScalarE, which is full there -- cast on DVE instead
                    nc.vector.tensor_copy(w8[:, c0: c0 + QB],
                                          wr[:, c0: c0 + QB])
                else:
                    nc.scalar.copy(w8[:, c0: c0 + QB], wr[:, c0: c0 + QB])

        def phase_mlp_h1(qb):
            st = state[qb]
            w8v = st["w8"][:].rearrange("p (t q) -> p t q", t=HT)
            wv = w3("wo1T")
            h1 = st_pool.tile([128, HT * QB], FP8, tag=f"h1_{qb}",
                              name=f"h1T{qb}")
            for otp in range(HT // 2):
                pair = ps_pair.tile([128, 2 * QB], dt.float32, tag="pair",
                                    name=f"ps_h1_{qb}_{otp}")
                for h in range(2):
                    ot = 2 * otp + h
                    for jo in range(HT // 2):
                        nc.tensor.matmul(
                            pair[:, h * QB:(h + 1) * QB],
                            wv[:, ot * HT + 2 * jo: ot * HT + 2 * jo + 2, :],
                            w8v[:, 2 * jo: 2 * jo + 2, :],
                            start=(jo == 0),
                            stop=(jo == HT // 2 - 1),
                            perf_mode=PMODE,
                        )
                for h in range(2):
                    ot = 2 * otp + h
                    nc.scalar.activation(
                        h1[:, ot * QB:(ot + 1) * QB],
                        pair[:, h * QB:(h + 1) * QB],
                        AF.Relu, bias=biases["bo1"][:, ot: ot + 1],
                        scale=1.0 / WO1_SCALE,
                    )
            st["h1"] = h1

        def phase_mlp_out(qb):
            """out = h1 @ Wo2^T + (w + bo2): PSUM + residual in one DVE add,
            straight to bf16 DMA staging."""
            st = state[qb]
            h1v = st["h1"][:].rearrange("p (t q) -> p t q", t=HT)
            wv = w3("wo2T")
            for otp in range(HT // 2):
                pair = ps_pair.tile([128, 2 * QB], dt.float32, tag="pair",
                                    name=f"ps_o_{qb}_{otp}")
                for h in range(2):
                    ot = 2 * otp + h
                    for jo in range(HT // 2):
                        nc.tensor.matmul(
                            pair[:, h * QB:(h + 1) * QB],
                            wv[:, ot * HT + 2 * jo: ot * HT + 2 * jo + 2, :],
                            h1v[:, 2 * jo: 2 * jo + 2, :],
                            start=(jo == 0),
                            stop=(jo == HT // 2 - 1),
                            perf_mode=PMODE,
                        )
                o_mid = out_pool.tile([128, 2 * QB], BF16, tag="o_mid",
                                      name=f"omid_{qb}_{otp}")
                o_sb = out_pool.tile([128, 2 * QB], BF16, tag="outT_blk",
                                     name=f"outT_{qb}_{otp}")
                c0 = (qb * HT + otp * 2) * QB
                # the very last block streams out in 512-wide chunks so the
                # act -> add -> DMA tail pipeline overlaps
                nchunk = 2 if (qb, otp) == (1, HT // 2 - 1) else 1
                for ch in range(nchunk):
                    w = 2 * QB // nchunk
                    sl = slice(ch * w, (ch + 1) * w)
                    nc.scalar.activation(o_mid[:, sl], pair[:, sl], AF.Identity,
                                         scale=1.0 / WO2_SCALE)
                    nc.vector.tensor_add(
                        o_sb[:, sl], o_mid[:, sl],
                        st["wr"][:, otp * 2 * QB + ch * w:
                                 otp * 2 * QB + (ch + 1) * w],
                    )
                    nc.sync.dma_start(
                        outT_ext[:, c0 + ch * w: c0 + (ch + 1) * w],
                        o_sb[:, sl],
                    )

        # software pipeline: DVE/ScalarE chains (norm, weighted, h1-acts) are
        # always covered by an independent PE phase emitted around them
        phase_scores(0)
        phase_pv_half(0, 0)
        phase_weighted_half(0, 0)
        phase_pv_half(0, 1)
        phase_weighted_half(0, 1)
        phase_scores(1)
        phase_mlp_h1(0)
        phase_pv_half(1, 0)
        phase_weighted_half(1, 0)
        phase_pv_half(1, 1)
        phase_weighted_half(1, 1)
        phase_mlp_out(0)
        phase_mlp_h1(1)
        phase_mlp_out(1)


# ---- host-side shard packing ----

def _tile_rows(a):
    """[T*128, N] -> [128, T*N]: partition-tiled T-layout, contiguous DMA."""
    t = a.shape[0] // 128
    return a.reshape(t, 128, a.shape[1]).transpose(1, 0, 2).reshape(128, -1)


def _tile_weight(w):
    """W^T [768h, 768o] -> [128, (ot, ht, 128)]: o-major packed lhsT tiles."""
    x = w.reshape(HT, 128, HT, 128)          # [ht, p, ot, o128]
    return x.transpose(1, 2, 0, 3).reshape(128, -1)


def _tile_rows_blocked(a, qb):
    """[768, NB*qb] -> [128, NB*(6*qb)]: per-block ht-major packing."""
    nb = a.shape[1] // qb
    x = a.reshape(HT, 128, nb, qb).transpose(1, 2, 0, 3)
    return x.reshape(128, -1)


def shard_inputs(query, key, value, Wq, bq, Wk, bk, Wo1, bo1, Wo2, bo2):
    """Full inputs -> per-core in_maps (host packing, fp8 cast, scale folds)."""
    scale = np.float32(1.0 / np.sqrt(np.float32(H)))

    def c8(x):
        return np.ascontiguousarray(
            np.clip(np.asarray(x, np.float32), -240, 240).astype(NP_FP8))

    def cb(x):
        return np.ascontiguousarray(np.asarray(x, np.float32).astype(NP_BF16))

    def cf(x):
        return np.ascontiguousarray(x.astype(np.float32))

    bo1p = bo1 - Wo1 @ bo2           # corrects for the +bo2 folded into w'
    shared = {
        "wqT": c8(_tile_weight(Wq.T * (scale * WQ_SCALE))),
        "wkT": c8(_tile_weight(Wk.T * WK_SCALE)),
        "wo1T": c8(_tile_weight(Wo1.T * WO1_SCALE)),
        "wo2T": c8(_tile_weight(Wo2.T * WO2_SCALE)),
        "biases": cf(np.concatenate([
            (bq * scale).reshape(HT, 128).T, bk.reshape(HT, 128).T,
            np.asarray(bo1p).reshape(HT, 128).T], axis=1)),
    }
    in_maps = []
    for core in range(N_CORES):
        b, half = divmod(core, 2)
        r0 = half * QCHUNK
        vTb = np.asarray(value[b]).T + np.asarray(bo2)[:, None]
        in_maps.append({
            "qT": c8(_tile_rows_blocked(query[b].T[:, r0: r0 + QCHUNK], QB)),
            "kT": c8(_tile_rows_blocked(key[b].T, QB)),
            "v": c8(_tile_rows(np.asarray(value[b]))),
            "vTb": cb(_tile_rows_blocked(vTb[:, r0: r0 + QCHUNK], QB)),
            **shared,
        })
    return in_maps


def gather_outputs(results):
    """Per-core outT [128, NQB*HT*QB] bf16 -> full [B, S, H] fp32."""
    out = np.empty((B, S, H), dtype=np.float32)
    for core in range(N_CORES):
        b, half = divmod(core, 2)
        r0 = half * QCHUNK
        buf = results[core]["outT"].reshape(128, NQB, HT, QB)
        # out[q0+qb*QB+n, ot*128+p] = buf[p, qb, ot, n]
        out[b, r0: r0 + QCHUNK] = (
            buf.transpose(1, 3, 2, 0).reshape(QCHUNK, H).astype(np.float32)
        )
    return out


def run(inputs, trace=False):
    nc = build_kernel()
    in_maps = shard_inputs(**{k: np.asarray(v) for k, v in inputs.items()})
    res = run_bass_kernel_spmd(nc, in_maps, list(range(N_CORES)), trace=trace)
    return gather_outputs(res.results), res


def _split_multi_waits(nc):
    """Workaround for this container's walrus rejecting instructions that
    carry more than one semaphore wait ("Too many sync wait commands"):
    hoist N-1 waits onto fresh single-wait same-engine InstNoOp instructions
    inserted immediately before the instruction. Engine streams execute the
    block's per-engine subsequence in order, so blocking on the nops first is
    semantically identical to one multi-wait instruction."""
    for f in nc.m.functions:
        for bb in f.blocks:
            insts = list(bb.instructions)
            out = []
            changed = False
            for inst in insts:
                si = inst.sync_info
                waits = list(si.on_wait) if si is not None and si.on_wait else []
                if len(waits) > 1:
                    changed = True
                    for w in waits[:-1]:
                        nop = mybir.InstNoOp(
                            name=nc.get_next_instruction_name(), ins=[], outs=[]
                        )
                        nop.engine = inst.engine
                        nop.sync_info = mybir.SyncInfo(on_wait=[w], on_update=[])
                        out.append(nop)
                    si.on_wait = waits[-1:]
                    inst.sync_info = si
                out.append(inst)
            if changed:
                bb.instructions = out


def kernel(**inputs):
    """Entry point: full (unsharded) numpy inputs -> full [B, S, H] output."""
    out, _ = run(inputs, trace=False)
    return out


# revision 39
# speedup vs baseline: 1.2305x; 1.0069x over previous
"""Distributed single-head attention + MLP block for 8 TRN2 NeuronCores.

Reference computation (per batch b):
  Q = query @ Wq^T + bq ; K = key @ Wk^T + bk
  scores = Q @ K^T / sqrt(H) ; attn = softmax(scores)
  weighted = attn @ value + value
  h1 = relu(weighted @ Wo1^T + bo1)
  out = h1 @ Wo2^T + bo2 + weighted

Sharding: B=4 batches x 2 query-row halves = 8 shards. Each core gets its
1024 query rows plus the full 2048 keys/values of its batch; attention is
dense (non-causal) so no inter-core communication is needed.

Layout strategy: everything on-device lives feature-on-partitions
("T-layout", X^T[f, tok]) so all matmul contractions line up with zero
on-device transposes, and the host pre-packs every shard into the exact
[128, free] SBUF tiling the kernel consumes. All five GEMMs run in fp8
DoubleRow (2 contraction rows/cycle); fp8 weight tensors are scaled up
x64/x256 on the host to escape the e4m3 subnormal range and un-scaled for
free via the ScalarE activation's scale argument. The softmax needs no
max-subtraction: scores have std ~1/3 by construction.

Residual/bias algebra: the host ships vTb = value^T + bo2, so the kernel's
"weighted + bo2" residual costs nothing; feeding the MLP with w' = w + bo2
is corrected by bo1' = bo1 - Wo1 @ bo2 (exact), which lets the second MLP
GEMM skip ScalarE entirely (one DVE add straight out of PSUM).

PSUM is laid out as 2x rotating [128,1024] bank-pairs (scores/projection/
MLP accumulators, evacuated by ONE wide ScalarE activation each) + 3
single banks for the PV accumulation (processed in two ht-halves) + 1
norm/warmup bank. The softmax denominator is a 4-op contiguous DVE fold
tree over the [128, 16*512] exp tile, then the usual ones-matmul rowsum +
reciprocal + PE-broadcast. A dozen throwaway matmuls run during the
initial DMA wait to trip the PE HAM clock-gate to 2.4 GHz before the real
GEMM stream starts.
"""

import contextlib

import numpy as np
import ml_dtypes

import concourse.bass as bass
import concourse.mybir as mybir
import concourse.tile as tile
from concourse.bass_utils import run_bass_kernel_spmd

dt = mybir.dt
AF = mybir.ActivationFunctionType

H = 768          # model dim
B = 4            # batch
S = 2048         # sequence length
N_CORES = 8
QCHUNK = S * B // N_CORES        # 1024 query rows per core
HT = H // 128                    # 6 feature partition-tiles
KTILES = S // 128                # 16 key partition-tiles
QB = 512                         # q-block width (= PSUM bank, fp32)
NQB = QCHUNK // QB               # 2 q-blocks per core

FP8 = dt.float8e4
NP_FP8 = dt.np(FP8)
BF16 = dt.bfloat16
NP_BF16 = ml_dtypes.bfloat16
PMODE = mybir.MatmulPerfMode.DoubleRow

WQ_SCALE = 256.0                 # host premultiplier on Wq (incl 1/sqrt(H))
WK_SCALE = 64.0                  # host premultiplier on Wk
WO1_SCALE = 64.0                 # host premultiplier on Wo1
WO2_SCALE = 64.0                 # host premultiplier on Wo2


def build_kernel():
    nc = bass.Bass()

    qT_ext = nc.declare_dram_parameter("qT", [128, HT * QCHUNK], FP8, isOutput=False)
    kT_ext = nc.declare_dram_parameter("kT", [128, HT * S], FP8, isOutput=False)
    v_ext = nc.declare_dram_parameter("v", [128, KTILES * H], FP8, isOutput=False)
    vTb_ext = nc.declare_dram_parameter("vTb", [128, HT * QCHUNK], BF16,
                                        isOutput=False)
    w_ext = {
        name: nc.declare_dram_parameter(name, [128, HT * H], FP8, isOutput=False)
        for name in ("wqT", "wkT", "wo1T", "wo2T")
    }
    b_ext = nc.declare_dram_parameter("biases", [128, 3 * HT], dt.float32,
                                      isOutput=False)
    outT_ext = nc.declare_dram_parameter(
        "outT", [128, HT * QCHUNK], BF16, isOutput=True
    )

    with tile.TileContext(nc) as tc, nc.allow_low_precision(
        reason="fp8 matmul path is intentional; rel-err budget is 2e-2"
    ):
        _body(nc, tc, qT_ext, kT_ext, v_ext, vTb_ext, w_ext, b_ext, outT_ext)

    _split_multi_waits(nc)
    return nc


def _body(nc, tc, qT_ext, kT_ext, v_ext, vTb_ext, w_ext, b_ext, outT_ext):
    with contextlib.ExitStack() as ctx:
        const_pool = ctx.enter_context(tc.tile_pool(name="const", bufs=1))
        w_pool = ctx.enter_context(tc.tile_pool(name="w", bufs=1))
        act_pool = ctx.enter_context(tc.tile_pool(name="act", bufs=1))
        st_pool = ctx.enter_context(tc.tile_pool(name="st", bufs=1))
        out_pool = ctx.enter_context(tc.tile_pool(name="outs", bufs=3))
        # PSUM: 2 x [128,1024] rotating bank-pairs + 3 PV banks + 1 norm bank.
        ps_pair = ctx.enter_context(tc.tile_pool(name="ps_pair", bufs=2,
                                                 space="PSUM"))
        ps_one = ctx.enter_context(tc.tile_pool(name="ps_one", bufs=1,
                                                space="PSUM"))

        # ---- constants + PE warm-up (no DMA dependency: runs during the
        # input DMA head and trips the HAM clock gate to full rate) ----
        wu = const_pool.tile([128, 256], BF16, tag="warmup")
        nc.vector.memset(wu[:], 0.002)
        ones_f32 = const_pool.tile([128, 128], dt.float32, tag="ones_f32")
        nc.vector.memset(ones_f32[:], 1.0)
        ones_row = const_pool.tile([1, 128], dt.float32r, tag="ones_row")
        nc.vector.tensor_copy(ones_row[:], ones_f32[0:1, :])
        ones_col = const_pool.tile([128, 1], dt.float32r, tag="ones_col")
        nc.vector.tensor_copy(ones_col[:], ones_f32[:, 0:1])
        # fp8 ones pair for the qb1 denominator matmul: DR lhsT needs the
        # k-tile step to be a multiple of 16 bytes, so cols 0 and 16 of a
        # 32-wide tile are the two "rows" the AP actually reads.
        ones8 = const_pool.tile([128, 32], FP8, tag="ones8")
        nc.vector.memset(ones8[:], 1.0)
        ones8v = ones8[:].rearrange("p (t m) -> p t m", t=2)[:, :, 0:1]
        ln64 = const_pool.tile([128, 1], dt.float32, tag="ln64")
        nc.vector.memset(ln64[:], float(np.log(WO2_SCALE)))

        # ---- input DMAs in first-use order. The first-matmul chain (kT
        # block A, wkT chunk 0, biases, rest of wkT, kT block B) rides the
        # sync queue; ScalarE's queue opens with a dummy Ln activation that
        # pre-pays the ~2.7us ACT table load while those DMAs fly (Ln selects
        # natural_log_exp_and_others, which also holds Exp/Relu/Identity/
        # Copy, so this is the only table load in the kernel). ----
        actwarm = const_pool.tile([1, 2], dt.float32, tag="actwarm")
        nc.scalar.activation(actwarm[:], ones_f32[0:1, 0:2], AF.Ln)

        kT_in = act_pool.tile([128, HT * S], FP8, tag="kT_in")
        nc.sync.dma_start(kT_in[:, : HT * QB * 2], kT_ext[:, : HT * QB * 2])
        w_sb = {
            name: w_pool.tile([128, HT * H], FP8, tag=name, name=f"w_{name}")
            for name in ("wqT", "wkT", "wo1T", "wo2T")
        }
        for c0, c1 in ((0, 1), (1, 2), (2, 3), (3, HT)):
            nc.sync.dma_start(w_sb["wkT"][:, c0 * HT * 128: c1 * HT * 128],
                              w_ext["wkT"][:, c0 * HT * 128: c1 * HT * 128])
        bias_sb = const_pool.tile([128, 3 * HT], dt.float32, tag="biases")
        nc.sync.dma_start(bias_sb[:], b_ext[:])
        biases = {name: bias_sb[:, i * HT:(i + 1) * HT]
                  for i, name in enumerate(("bq", "bk", "bo1"))}
        nc.sync.dma_start(kT_in[:, HT * QB * 2:], kT_ext[:, HT * QB * 2:])
        nc.scalar.dma_start(w_sb["wqT"][:], w_ext["wqT"][:])
        qT_in = act_pool.tile([128, HT * QCHUNK], FP8, tag="qT_in")
        nc.scalar.dma_start(qT_in[:], qT_ext[:])
        v_sb = act_pool.tile([128, KTILES * H], FP8, tag="v_in")
        nc.sync.dma_start(v_sb[:], v_ext[:])
        nc.sync.dma_start(w_sb["wo1T"][:], w_ext["wo1T"][:])
        nc.sync.dma_start(w_sb["wo2T"][:], w_ext["wo2T"][:])
        vTb_in = act_pool.tile([128, HT * QCHUNK], BF16, tag="vTb_in")
        nc.sync.dma_start(vTb_in[:], vTb_ext[:])

        # warm-up matmuls (~2.2us of junk PE work into the norm bank)
        for i in range(10):
            ps_wu = ps_one.tile([128, QB], dt.float32, tag="norm",
                                name=f"wu{i}")
            nc.tensor.matmul(ps_wu[:, :256], wu[:, :128], wu[:],
                             start=True, stop=True)

        def w3(name):
            return w_sb[name][:].rearrange("p (o t m) -> p (o t) m", o=HT, t=HT)

        # ---- K/Q projections: jo-contraction into [128,1024] bank-pairs,
        # one wide ScalarE activation per (ot, block-pair) ----
        def project(wname, bias, scale, x_in, nblocks, out_sb, out_col, tag,
                    head_split=False):
            wv = w3(wname)
            xv = x_in[:].rearrange("p (n t q) -> p n t q", n=nblocks, t=HT)
            head_banks = {(0, 0): "pvw0", (0, 1): "pvw1",
                          (1, 0): "pvw2", (1, 1): "norm"}
            for nb2 in range(nblocks // 2):
                for ot in range(HT):
                    # the very first evacuations ride 4 idle single banks so
                    # the act-latency ramp doesn't stall the bank-pair pool
                    split = head_split and nb2 == 0 and ot < 2
                    if split:
                        tiles = [ps_one.tile([128, QB], dt.float32,
                                             tag=head_banks[(ot, h)],
                                             name=f"ps_{tag}h_{ot}_{h}")
                                 for h in range(2)]
                        tgt = lambda h: tiles[h][:]
                    else:
                        pair = ps_pair.tile([128, 2 * QB], dt.float32,
                                            tag="pair",
                                            name=f"ps_{tag}_{nb2}_{ot}")
                        tgt = lambda h: pair[:, h * QB:(h + 1) * QB]
                    for jo in range(HT // 2):
                        for h in range(2):
                            nc.tensor.matmul(
                                tgt(h),
                                wv[:, ot * HT + 2 * jo: ot * HT + 2 * jo + 2, :],
                                xv[:, 2 * nb2 + h, 2 * jo: 2 * jo + 2, :],
                                start=(jo == 0),
                                stop=(jo == HT // 2 - 1),
                                perf_mode=PMODE,
                            )
                    c0 = out_col(ot, nb2)
                    if split:
                        for h in range(2):
                            nc.scalar.activation(
                                out_sb[:, c0 + h * QB: c0 + (h + 1) * QB],
                                tiles[h][:], AF.Identity,
                                bias=bias[:, ot: ot + 1], scale=scale,
                            )
                    else:
                        nc.scalar.activation(
                            out_sb[:, c0: c0 + 2 * QB], pair[:], AF.Identity,
                            bias=bias[:, ot: ot + 1], scale=scale,
                        )

        KT = act_pool.tile([128, HT * S], FP8, tag="KT", name="KT_full")
        project("wkT", biases["bk"], 1.0 / WK_SCALE, kT_in, 4, KT,
                lambda ot, nb2: ot * S + nb2 * 2 * QB, "KT", head_split=True)
        QT = act_pool.tile([128, HT * QCHUNK], FP8, tag="QT", name="proj_QT")
        project("wqT", biases["bq"], 1.0 / WQ_SCALE, qT_in, 2, QT,
                lambda ot, nb2: ot * QCHUNK + nb2 * 2 * QB, "QT")
        KT3 = KT[:].rearrange("p (t k) -> p t k", t=HT)
        QT3 = QT[:].rearrange("p (t q) -> p t q", t=HT)

        def vpair(jk, ht):
            """lhsT [128, 2, 128]: k-tile pair (2jk, 2jk+1), h-tile ht."""
            return (v_sb[:].rearrange("p (t h) -> p t h", t=KTILES)
                    [:, 2 * jk: 2 * jk + 2, ht * 128:(ht + 1) * 128])

        # ---- attention + MLP, software-pipelined across q-blocks ----
        state = {}

        def den_mm(qb, p8):
            """qb1 path: accumulate exp-pair p8 into the [1,512] rowsum via
            a ones matmul (contraction over 128 partitions x 2 k-tiles)."""
            st = state[qb]
            rhs8 = st["expT"][:].rearrange("p (j t q) -> p j t q",
                                           j=KTILES // 2, t=2)
            nc.tensor.matmul(
                st["ps_den"][0:1, :], ones8v, rhs8[:, p8],
                start=(p8 == 0), stop=(p8 == KTILES // 2 - 1),
                perf_mode=PMODE,
            )

        def phase_scores(qb):
            """scores^T + exp, two k-tiles per PSUM bank-pair / ACTIVATE.
            Denominator: qb0 accumulates on the (idle there) DVE as
            incremental [128,1024] adds; qb1's scores window already has
            weighted-qb0 DVE work, so its denominator rides the PE as tiny
            ones-matmuls trailing one pair behind the exps."""
            q0 = qb * QB
            expT = st_pool.tile([128, KTILES * QB], FP8, tag=f"expT{qb}",
                                name=f"expT{qb}")
            state[qb] = {"expT": expT}
            if qb == 0:
                acc = st_pool.tile([128, 2 * QB], BF16, tag="acc0")
            else:
                state[qb]["ps_den"] = ps_one.tile(
                    [128, QB], dt.float32, tag="norm", name="ps_den1")
            for p8 in range(KTILES // 2):
                pair = ps_pair.tile([128, 2 * QB], dt.float32, tag="pair",
                                    name=f"ps_s_{qb}_{p8}")
                for half in range(2):
                    kt = 2 * p8 + half
                    for jo in range(HT // 2):
                        nc.tensor.matmul(
                            pair[:, half * QB:(half + 1) * QB],
                            KT3[:, 2 * jo: 2 * jo + 2, kt * 128:(kt + 1) * 128],
                            QT3[:, 2 * jo: 2 * jo + 2, q0: q0 + QB],
                            start=(jo == 0),
                            stop=(jo == HT // 2 - 1),
                            perf_mode=PMODE,
                        )
                sl = expT[:, p8 * 2 * QB:(p8 + 1) * 2 * QB]
                nc.scalar.activation(sl, pair[:], AF.Exp)
                if qb == 0:
                    if p8 == 0:
                        nc.vector.tensor_copy(acc[:], sl)
                    else:
                        nc.vector.tensor_add(acc[:], acc[:], sl)
                elif p8 >= 1:
                    den_mm(qb, p8 - 1)
            if qb == 0:
                sum_part = st_pool.tile([128, QB], dt.float32r, tag="sump0")
                nc.vector.tensor_add(sum_part[:], acc[:, :QB], acc[:, QB:])
                state[qb]["sum_part"] = sum_part

        def phase_norm_ln(qb):
            """Partition-reduce the rowsum (qb0: one ones-matmul on the DVE
            partial; qb1: close the PE den group), then 1/rowsum =
            exp(-ln(rowsum)) on ScalarE: no DVE reciprocal."""
            st = state[qb]
            if qb == 0:
                ps_den = ps_one.tile([128, QB], dt.float32, tag="norm",
                                     name="ps_den0")
                nc.tensor.matmul(ps_den[0:1, :], ones_col[:],
                                 st["sum_part"][:], start=True, stop=True)
            else:
                den_mm(qb, KTILES // 2 - 1)
                ps_den = st["ps_den"]
            logsum = st_pool.tile([1, QB], dt.float32r, tag="logsum",
                                  name=f"logsum{qb}")
            nc.scalar.activation(logsum[:], ps_den[0:1, :], AF.Ln)
            st["logsum"] = logsum

        def phase_norm_bcast(qb):
            """bcast64 = exp(ln64 - ln(rowsum)) = 64/rowsum: the x64 keeps
            the weighted residual in the same scale as the x64 Wo2 PSUM so
            the MLP output evacuates in ONE fused DVE op."""
            st = state[qb]
            ps_b = ps_one.tile([128, QB], dt.float32, tag="norm",
                               name=f"ps_b{qb}")
            nc.tensor.matmul(ps_b[:], ones_row[:], st["logsum"][:],
                             start=True, stop=True)
            bcast = st_pool.tile([128, QB], dt.float32, tag="bcast",
                                 name=f"bcast{qb}")
            nc.scalar.activation(bcast[:], ps_b[:], AF.Exp, scale=-1.0)
            st["bcast"] = bcast

        def phase_pv_half(qb, half):
            """PV for 3 h-tiles over all 16 k-tiles; norm chain of this
            q-block interleaves under half 0."""
            st = state[qb]
            rhs8 = st["expT"][:].rearrange("p (j t q) -> p j t q",
                                           j=KTILES // 2, t=2)
            ps_w = [ps_one.tile([128, QB], dt.float32, tag=f"pvw{i}",
                                name=f"pvw{i}_{qb}_{half}")
                    for i in range(3)]
            if half == 0:
                sched = [(jk, i) for jk in range(KTILES // 2) for i in range(3)]
            else:
                # skewed wavefront: bank i starts i waves late, so the first
                # matmul of each bank lands just after the previous half's
                # weighted-mul releases that bank (no lump wait on DVE)
                sched = [(w - i, i) for w in range(KTILES // 2 + 2)
                         for i in range(3) if 0 <= w - i < KTILES // 2]
            for n, (jk, i) in enumerate(sched):
                if half == 0 and (jk, i) == (1, 0):
                    phase_norm_ln(qb)
                if half == 0 and (jk, i) == (3, 0):
                    phase_norm_bcast(qb)
                nc.tensor.matmul(
                    ps_w[i][:],
                    vpair(jk, 3 * half + i),
                    rhs8[:, jk],
                    start=(jk == 0),
                    stop=(jk == KTILES // 2 - 1),
                    perf_mode=PMODE,
                )
            st[f"ps_w{half}"] = ps_w

        def phase_weighted_half(qb, half):
            """w = PV/rowsum + (value^T + bo2); bf16 residual + fp8 GEMM copy."""
            st = state[qb]
            ps_w = st[f"ps_w{half}"]
            if "wr" not in st:
                st["wr"] = st_pool.tile([128, HT * QB], BF16, tag=f"wr{qb}",
                                        name=f"wr{qb}")
                st["w8"] = st_pool.tile([128, HT * QB], FP8, tag=f"w8_{qb}",
                                        name=f"w8_{qb}")
            wr, w8 = st["wr"], st["w8"]
            for i in range(3):
                ht = 3 * half + i
                c0 = ht * QB
                tmp = out_pool.tile([128, QB], dt.float32, tag="wtmp",
                                    name=f"wtmp_{qb}_{ht}")
                nc.vector.tensor_mul(tmp[:], ps_w[i][:], st["bcast"][:])
                nc.vector.tensor_add(
                    wr[:, c0: c0 + QB], tmp[:],
                    vTb_in[:, qb * HT * QB + c0: qb * HT * QB + c0 + QB],
                )
                if (qb, half) == (0, 1):
                    # this cast's consumers sit behind scores1's exps on
                    # ScalarE, which is full there -- cast on DVE instead
                    nc.vector.tensor_copy(w8[:, c0: c0 + QB],
                                          wr[:, c0: c0 + QB])
                else:
                    nc.scalar.copy(w8[:, c0: c0 + QB], wr[:, c0: c0 + QB])

        def phase_mlp_h1(qb):
            st = state[qb]
            w8v = st["w8"][:].rearrange("p (t q) -> p t q", t=HT)
            wv = w3("wo1T")
            h1 = st_pool.tile([128, HT * QB], FP8, tag=f"h1_{qb}",
                              name=f"h1T{qb}")
            for otp in range(HT // 2):
                pair = ps_pair.tile([128, 2 * QB], dt.float32, tag="pair",
                                    name=f"ps_h1_{qb}_{otp}")
                for h in range(2):
                    ot = 2 * otp + h
                    for jo in range(HT // 2):
                        nc.tensor.matmul(
                            pair[:, h * QB:(h + 1) * QB],
                            wv[:, ot * HT + 2 * jo: ot * HT + 2 * jo + 2, :],
                            w8v[:, 2 * jo: 2 * jo + 2, :],
                            start=(jo == 0),
                            stop=(jo == HT // 2 - 1),
                            perf_mode=PMODE,
                        )
                for h in range(2):
                    ot = 2 * otp + h
                    nc.scalar.activation(
                        h1[:, ot * QB:(ot + 1) * QB],
                        pair[:, h * QB:(h + 1) * QB],
                        AF.Relu, bias=biases["bo1"][:, ot: ot + 1],
                        scale=1.0 / WO1_SCALE,
                    )
            st["h1"] = h1

        def phase_mlp_out(qb):
            """out = h1 @ Wo2^T + (w + bo2): PSUM + residual in one DVE add,
            straight to bf16 DMA staging."""
            st = state[qb]
            h1v = st["h1"][:].rearrange("p (t q) -> p t q", t=HT)
            wv = w3("wo2T")
            for otp in range(HT // 2):
                pair = ps_pair.tile([128, 2 * QB], dt.float32, tag="pair",
                                    name=f"ps_o_{qb}_{otp}")
                for h in range(2):
                    ot = 2 * otp + h
                    for jo in range(HT // 2):
                        nc.tensor.matmul(
                            pair[:, h * QB:(h + 1) * QB],
                            wv[:, ot * HT + 2 * jo: ot * HT + 2 * jo + 2, :],
                            h1v[:, 2 * jo: 2 * jo + 2, :],
                            start=(jo == 0),
                            stop=(jo == HT // 2 - 1),
                            perf_mode=PMODE,
                        )
                o_mid = out_pool.tile([128, 2 * QB], BF16, tag="o_mid",
                                      name=f"omid_{qb}_{otp}")
                o_sb = out_pool.tile([128, 2 * QB], BF16, tag="outT_blk",
                                     name=f"outT_{qb}_{otp}")
                c0 = (qb * HT + otp * 2) * QB
                # the very last block streams out in 512-wide chunks so the
                # act -> add -> DMA tail pipeline overlaps
                nchunk = 2 if (qb, otp) == (1, HT // 2 - 1) else 1
                for ch in range(nchunk):
                    w = 2 * QB // nchunk
                    sl = slice(ch * w, (ch + 1) * w)
                    nc.scalar.activation(o_mid[:, sl], pair[:, sl], AF.Identity,
                                         scale=1.0 / WO2_SCALE)
                    nc.vector.tensor_add(
                        o_sb[:, sl], o_mid[:, sl],
                        st["wr"][:, otp * 2 * QB + ch * w:
                                 otp * 2 * QB + (ch + 1) * w],
                    )
                    nc.sync.dma_start(
                        outT_ext[:, c0 + ch * w: c0 + (ch + 1) * w],
                        o_sb[:, sl],
                    )

        # software pipeline: DVE/ScalarE chains (norm, weighted, h1-acts) are
        # always covered by an independent PE phase emitted around them
        phase_scores(0)
        phase_pv_half(0, 0)
        phase_weighted_half(0, 0)
        phase_pv_half(0, 1)
        phase_weighted_half(0, 1)
        phase_scores(1)
        phase_mlp_h1(0)
        phase_pv_half(1, 0)
        phase_weighted_half(1, 0)
        phase_pv_half(1, 1)
        phase_weighted_half(1, 1)
        phase_mlp_out(0)
        phase_mlp_h1(1)
        phase_mlp_out(1)


# ---- host-side shard packing ----

def _tile_rows(a):
    """[T*128, N] -> [128, T*N]: partition-tiled T-layout, contiguous DMA."""
    t = a.shape[0] // 128
    return a.reshape(t, 128, a.shape[1]).transpose(1, 0, 2).reshape(128, -1)


def _tile_weight(w):
    """W^T [768h, 768o] -> [128, (ot, ht, 128)]: o-major packed lhsT tiles."""
    x = w.reshape(HT, 128, HT, 128)          # [ht, p, ot, o128]
    return x.transpose(1, 2, 0, 3).reshape(128, -1)


def _tile_rows_blocked(a, qb):
    """[768, NB*qb] -> [128, NB*(6*qb)]: per-block ht-major packing."""
    nb = a.shape[1] // qb
    x = a.reshape(HT, 128, nb, qb).transpose(1, 2, 0, 3)
    return x.reshape(128, -1)


def shard_inputs(query, key, value, Wq, bq, Wk, bk, Wo1, bo1, Wo2, bo2):
    """Full inputs -> per-core in_maps (host packing, fp8 cast, scale folds)."""
    scale = np.float32(1.0 / np.sqrt(np.float32(H)))

    def c8(x):
        return np.ascontiguousarray(
            np.clip(np.asarray(x, np.float32), -240, 240).astype(NP_FP8))

    def cb(x):
        return np.ascontiguousarray(np.asarray(x, np.float32).astype(NP_BF16))

    def cf(x):
        return np.ascontiguousarray(x.astype(np.float32))

    bo1p = bo1 - Wo1 @ bo2           # corrects for the +bo2 folded into w'
    shared = {
        "wqT": c8(_tile_weight(Wq.T * (scale * WQ_SCALE))),
        "wkT": c8(_tile_weight(Wk.T * WK_SCALE)),
        "wo1T": c8(_tile_weight(Wo1.T * WO1_SCALE)),
        "wo2T": c8(_tile_weight(Wo2.T * WO2_SCALE)),
        "biases": cf(np.concatenate([
            (bq * scale).reshape(HT, 128).T, bk.reshape(HT, 128).T,
            np.asarray(bo1p).reshape(HT, 128).T], axis=1)),
    }
    in_maps = []
    for core in range(N_CORES):
        b, half = divmod(core, 2)
        r0 = half * QCHUNK
        vTb = np.asarray(value[b]).T + np.asarray(bo2)[:, None]
        in_maps.append({
            "qT": c8(_tile_rows_blocked(query[b].T[:, r0: r0 + QCHUNK], QB)),
            "kT": c8(_tile_rows_blocked(key[b].T, QB)),
            "v": c8(_tile_rows(np.asarray(value[b]))),
            "vTb": cb(_tile_rows_blocked(vTb[:, r0: r0 + QCHUNK], QB)),
            **shared,
        })
    return in_maps


def gather_outputs(results):
    """Per-core outT [128, NQB*HT*QB] bf16 -> full [B, S, H] fp32."""
    out = np.empty((B, S, H), dtype=np.float32)
    for core in range(N_CORES):
        b, half = divmod(core, 2)
        r0 = half * QCHUNK
        buf = results[core]["outT"].reshape(128, NQB, HT, QB)
        # out[q0+qb*QB+n, ot*128+p] = buf[p, qb, ot, n]
        out[b, r0: r0 + QCHUNK] = (
            buf.transpose(1, 3, 2, 0).reshape(QCHUNK, H).astype(np.float32)
        )
    return out


def run(inputs, trace=False):
    nc = build_kernel()
    in_maps = shard_inputs(**{k: np.asarray(v) for k, v in inputs.items()})
    res = run_bass_kernel_spmd(nc, in_maps, list(range(N_CORES)), trace=trace)
    return gather_outputs(res.results), res


def _split_multi_waits(nc):
    """Workaround for this container's walrus rejecting instructions that
    carry more than one semaphore wait ("Too many sync wait commands"):
    hoist N-1 waits onto fresh single-wait same-engine InstNoOp instructions
    inserted immediately before the instruction. Engine streams execute the
    block's per-engine subsequence in order, so blocking on the nops first is
    semantically identical to one multi-wait instruction."""
    for f in nc.m.functions:
        for bb in f.blocks:
            insts = list(bb.instructions)
            out = []
            changed = False
            for inst in insts:
                si = inst.sync_info
                waits = list(si.on_wait) if si is not None and si.on_wait else []
                if len(waits) > 1:
                    changed = True
                    for w in waits[:-1]:
                        nop = mybir.InstNoOp(
                            name=nc.get_next_instruction_name(), ins=[], outs=[]
                        )
                        nop.engine = inst.engine
                        nop.sync_info = mybir.SyncInfo(on_wait=[w], on_update=[])
                        out.append(nop)
                    si.on_wait = waits[-1:]
                    inst.sync_info = si
                out.append(inst)
            if changed:
                bb.instructions = out


def kernel(**inputs):
    """Entry point: full (unsharded) numpy inputs -> full [B, S, H] output."""
    out, _ = run(inputs, trace=False)
    return out


# revision 46
# speedup vs baseline: 1.2323x; 1.0015x over previous
"""Distributed single-head attention + MLP block for 8 TRN2 NeuronCores.

Reference computation (per batch b):
  Q = query @ Wq^T + bq ; K = key @ Wk^T + bk
  scores = Q @ K^T / sqrt(H) ; attn = softmax(scores)
  weighted = attn @ value + value
  h1 = relu(weighted @ Wo1^T + bo1)
  out = h1 @ Wo2^T + bo2 + weighted

Sharding: B=4 batches x 2 query-row halves = 8 shards. Each core gets its
1024 query rows plus the full 2048 keys/values of its batch; attention is
dense (non-causal) so no inter-core communication is needed.

Layout strategy: everything on-device lives feature-on-partitions
("T-layout", X^T[f, tok]) so all matmul contractions line up with zero
on-device transposes, and the host pre-packs every shard into the exact
[128, free] SBUF tiling the kernel consumes. All five GEMMs run in fp8
DoubleRow (2 contraction rows/cycle); fp8 weight tensors are scaled up
x64/x256 on the host to escape the e4m3 subnormal range and un-scaled for
free via the ScalarE activation's scale argument. The softmax needs no
max-subtraction: scores have std ~1/3 by construction.

Residual/bias algebra: the host ships vTb = value^T + bo2, so the kernel's
"weighted + bo2" residual costs nothing; feeding the MLP with w' = w + bo2
is corrected by bo1' = bo1 - Wo1 @ bo2 (exact), which lets the second MLP
GEMM skip ScalarE entirely (one DVE add straight out of PSUM).

PSUM is laid out as 2x rotating [128,1024] bank-pairs (scores/projection/
MLP accumulators, evacuated by ONE wide ScalarE activation each) + 3
single banks for the PV accumulation (processed in two ht-halves) + 1
norm/warmup bank. The softmax denominator is a 4-op contiguous DVE fold
tree over the [128, 16*512] exp tile, then the usual ones-matmul rowsum +
reciprocal + PE-broadcast. A dozen throwaway matmuls run during the
initial DMA wait to trip the PE HAM clock-gate to 2.4 GHz before the real
GEMM stream starts.
"""

import contextlib

import numpy as np
import ml_dtypes

import concourse.bass as bass
import concourse.mybir as mybir
import concourse.tile as tile
from concourse.bass_utils import run_bass_kernel_spmd

dt = mybir.dt
AF = mybir.ActivationFunctionType

H = 768          # model dim
B = 4            # batch
S = 2048         # sequence length
N_CORES = 8
QCHUNK = S * B // N_CORES        # 1024 query rows per core
HT = H // 128                    # 6 feature partition-tiles
KTILES = S // 128                # 16 key partition-tiles
QB = 512                         # q-block width (= PSUM bank, fp32)
NQB = QCHUNK // QB               # 2 q-blocks per core

FP8 = dt.float8e4
NP_FP8 = dt.np(FP8)
BF16 = dt.bfloat16
NP_BF16 = ml_dtypes.bfloat16
PMODE = mybir.MatmulPerfMode.DoubleRow

WQ_SCALE = 256.0                 # host premultiplier on Wq (incl 1/sqrt(H))
WK_SCALE = 64.0                  # host premultiplier on Wk
WO1_SCALE = 64.0                 # host premultiplier on Wo1
WO2_SCALE = 64.0                 # host premultiplier on Wo2


def build_kernel():
    nc = bass.Bass()

    qT_ext = nc.declare_dram_parameter("qT", [128, HT * QCHUNK], FP8, isOutput=False)
    kT_ext = nc.declare_dram_parameter("kT", [128, HT * S], FP8, isOutput=False)
    v_ext = nc.declare_dram_parameter("v", [128, KTILES * H], FP8, isOutput=False)
    vTb_ext = nc.declare_dram_parameter("vTb", [128, HT * QCHUNK], BF16,
                                        isOutput=False)
    w_ext = {
        name: nc.declare_dram_parameter(name, [128, HT * H], FP8, isOutput=False)
        for name in ("wqT", "wkT", "wo1T", "wo2T")
    }
    b_ext = nc.declare_dram_parameter("biases", [128, 3 * HT], dt.float32,
                                      isOutput=False)
    outT_ext = nc.declare_dram_parameter(
        "outT", [128, HT * QCHUNK], BF16, isOutput=True
    )

    with tile.TileContext(nc) as tc, nc.allow_low_precision(
        reason="fp8 matmul path is intentional; rel-err budget is 2e-2"
    ):
        _body(nc, tc, qT_ext, kT_ext, v_ext, vTb_ext, w_ext, b_ext, outT_ext)

    _split_multi_waits(nc)
    return nc


def _body(nc, tc, qT_ext, kT_ext, v_ext, vTb_ext, w_ext, b_ext, outT_ext):
    with contextlib.ExitStack() as ctx:
        const_pool = ctx.enter_context(tc.tile_pool(name="const", bufs=1))
        w_pool = ctx.enter_context(tc.tile_pool(name="w", bufs=1))
        act_pool = ctx.enter_context(tc.tile_pool(name="act", bufs=1))
        st_pool = ctx.enter_context(tc.tile_pool(name="st", bufs=1))
        out_pool = ctx.enter_context(tc.tile_pool(name="outs", bufs=3))
        # PSUM: 2 x [128,1024] rotating bank-pairs + 3 PV banks + 1 norm bank.
        ps_pair = ctx.enter_context(tc.tile_pool(name="ps_pair", bufs=2,
                                                 space="PSUM"))
        ps_one = ctx.enter_context(tc.tile_pool(name="ps_one", bufs=1,
                                                space="PSUM"))

        # ---- constants + PE warm-up (no DMA dependency: runs during the
        # input DMA head and trips the HAM clock gate to full rate) ----
        wu = const_pool.tile([128, 256], BF16, tag="warmup")
        nc.vector.memset(wu[:], 0.002)
        ones_f32 = const_pool.tile([128, 128], dt.float32, tag="ones_f32")
        nc.vector.memset(ones_f32[:], 1.0)
        ones_row = const_pool.tile([1, 128], dt.float32r, tag="ones_row")
        nc.vector.tensor_copy(ones_row[:], ones_f32[0:1, :])
        ones_col = const_pool.tile([128, 1], dt.float32r, tag="ones_col")
        nc.vector.tensor_copy(ones_col[:], ones_f32[:, 0:1])
        # fp8 ones pair for the qb1 denominator matmul: DR lhsT needs the
        # k-tile step to be a multiple of 16 bytes, so cols 0 and 16 of a
        # 32-wide tile are the two "rows" the AP actually reads.
        ones8 = const_pool.tile([128, 32], FP8, tag="ones8")
        nc.vector.memset(ones8[:], 1.0)
        ones8v = ones8[:].rearrange("p (t m) -> p t m", t=2)[:, :, 0:1]
        ln64 = const_pool.tile([128, 1], dt.float32, tag="ln64")
        nc.vector.memset(ln64[:], float(np.log(WO2_SCALE)))

        # ---- input DMAs in first-use order. The first-matmul chain (kT
        # block A, wkT chunk 0, biases, rest of wkT, kT block B) rides the
        # sync queue; ScalarE's queue opens with a dummy Ln activation that
        # pre-pays the ~2.7us ACT table load while those DMAs fly (Ln selects
        # natural_log_exp_and_others, which also holds Exp/Relu/Identity/
        # Copy, so this is the only table load in the kernel). ----
        # wkT chunks ride the scalar HWDGE queue (parallel with kT on sync),
        # issued BEFORE the dummy Ln act so the table load doesn't delay them
        w_sb = {
            name: w_pool.tile([128, HT * H], FP8, tag=name, name=f"w_{name}")
            for name in ("wqT", "wkT", "wo1T", "wo2T")
        }
        for c0, c1 in ((0, 1), (1, 2), (2, 3), (3, HT)):
            nc.scalar.dma_start(w_sb["wkT"][:, c0 * HT * 128: c1 * HT * 128],
                                w_ext["wkT"][:, c0 * HT * 128: c1 * HT * 128])
        actwarm = const_pool.tile([1, 2], dt.float32, tag="actwarm")
        nc.scalar.activation(actwarm[:], ones_f32[0:1, 0:2], AF.Ln)

        # kT block A lands in contraction order (ht tiles 0-3 of both
        # sub-blocks first) so the first projection matmuls start sooner
        kT_in = act_pool.tile([128, HT * S], FP8, tag="kT_in")
        C2 = 4 * 512
        for a, b in ((0, C2), (3072, 3072 + C2), (C2, 3072), (3072 + C2, 6144)):
            nc.sync.dma_start(kT_in[:, a:b], kT_ext[:, a:b])
        bias_sb = const_pool.tile([128, 3 * HT], dt.float32, tag="biases")
        nc.sync.dma_start(bias_sb[:], b_ext[:])
        biases = {name: bias_sb[:, i * HT:(i + 1) * HT]
                  for i, name in enumerate(("bq", "bk", "bo1"))}
        nc.sync.dma_start(kT_in[:, HT * QB * 2:], kT_ext[:, HT * QB * 2:])
        nc.scalar.dma_start(w_sb["wqT"][:], w_ext["wqT"][:])
        qT_in = act_pool.tile([128, HT * QCHUNK], FP8, tag="qT_in")
        nc.sync.dma_start(qT_in[:], qT_ext[:])
        v_sb = act_pool.tile([128, KTILES * H], FP8, tag="v_in")
        nc.sync.dma_start(v_sb[:], v_ext[:])
        nc.sync.dma_start(w_sb["wo1T"][:], w_ext["wo1T"][:])
        nc.sync.dma_start(w_sb["wo2T"][:], w_ext["wo2T"][:])
        vTb_in = act_pool.tile([128, HT * QCHUNK], BF16, tag="vTb_in")
        nc.sync.dma_start(vTb_in[:], vTb_ext[:])

        # warm-up matmuls (~1.5us of junk PE work into the norm bank)
        for i in range(7):
            ps_wu = ps_one.tile([128, QB], dt.float32, tag="norm",
                                name=f"wu{i}")
            nc.tensor.matmul(ps_wu[:, :256], wu[:, :128], wu[:],
                             start=True, stop=True)

        def w3(name):
            return w_sb[name][:].rearrange("p (o t m) -> p (o t) m", o=HT, t=HT)

        # ---- K/Q projections: jo-contraction into [128,1024] bank-pairs,
        # one wide ScalarE activation per (ot, block-pair) ----
        def project(wname, bias, scale, x_in, nblocks, out_sb, out_col, tag,
                    head_split=False):
            wv = w3(wname)
            xv = x_in[:].rearrange("p (n t q) -> p n t q", n=nblocks, t=HT)
            head_banks = {(0, 0): "pvw0", (0, 1): "pvw1",
                          (1, 0): "pvw2", (1, 1): "norm"}
            deferred = []
            for nb2 in range(nblocks // 2):
                for ot in range(HT):
                    # the very first evacuations ride 4 idle single banks so
                    # the act-latency ramp doesn't stall the bank-pair pool
                    split = head_split and nb2 == 0 and ot < 2
                    if split:
                        tiles = [ps_one.tile([128, QB], dt.float32,
                                             tag=head_banks[(ot, h)],
                                             name=f"ps_{tag}h_{ot}_{h}")
                                 for h in range(2)]
                        tgt = lambda h: tiles[h][:]
                    else:
                        pair = ps_pair.tile([128, 2 * QB], dt.float32,
                                            tag="pair",
                                            name=f"ps_{tag}_{nb2}_{ot}")
                        tgt = lambda h: pair[:, h * QB:(h + 1) * QB]
                    for jo in range(HT // 2):
                        for h in range(2):
                            nc.tensor.matmul(
                                tgt(h),
                                wv[:, ot * HT + 2 * jo: ot * HT + 2 * jo + 2, :],
                                xv[:, 2 * nb2 + h, 2 * jo: 2 * jo + 2, :],
                                start=(jo == 0),
                                stop=(jo == HT // 2 - 1),
                                perf_mode=PMODE,
                            )
                    c0 = out_col(ot, nb2)
                    if split:
                        # defer these evacuations: their banks aren't needed
                        # until PV, and emitting them later keeps the first
                        # NEEDED act (ot2's, gating ot4) at the queue front
                        deferred.append((c0, tiles, bias[:, ot: ot + 1]))
                    else:
                        nc.scalar.activation(
                            out_sb[:, c0: c0 + 2 * QB], pair[:], AF.Identity,
                            bias=bias[:, ot: ot + 1], scale=scale,
                        )
                        if deferred and ot == 3:
                            for d0, dtiles, db in deferred:
                                for h in range(2):
                                    nc.scalar.activation(
                                        out_sb[:, d0 + h * QB:
                                               d0 + (h + 1) * QB],
                                        dtiles[h][:], AF.Identity,
                                        bias=db, scale=scale,
                                    )
                            deferred = []

        KT = act_pool.tile([128, HT * S], FP8, tag="KT", name="KT_full")
        project("wkT", biases["bk"], 1.0 / WK_SCALE, kT_in, 4, KT,
                lambda ot, nb2: ot * S + nb2 * 2 * QB, "KT", head_split=True)
        QT = act_pool.tile([128, HT * QCHUNK], FP8, tag="QT", name="proj_QT")
        project("wqT", biases["bq"], 1.0 / WQ_SCALE, qT_in, 2, QT,
                lambda ot, nb2: ot * QCHUNK + nb2 * 2 * QB, "QT")
        KT3 = KT[:].rearrange("p (t k) -> p t k", t=HT)
        QT3 = QT[:].rearrange("p (t q) -> p t q", t=HT)

        def vpair(jk, ht):
            """lhsT [128, 2, 128]: k-tile pair (2jk, 2jk+1), h-tile ht."""
            return (v_sb[:].rearrange("p (t h) -> p t h", t=KTILES)
                    [:, 2 * jk: 2 * jk + 2, ht * 128:(ht + 1) * 128])

        # ---- attention + MLP, software-pipelined across q-blocks ----
        state = {}

        def den_mm(qb, p8):
            """qb1 path: accumulate exp-pair p8 into the [1,512] rowsum via
            a ones matmul (contraction over 128 partitions x 2 k-tiles)."""
            st = state[qb]
            rhs8 = st["expT"][:].rearrange("p (j t q) -> p j t q",
                                           j=KTILES // 2, t=2)
            nc.tensor.matmul(
                st["ps_den"][0:1, :], ones8v, rhs8[:, p8],
                start=(p8 == 0), stop=(p8 == KTILES // 2 - 1),
                perf_mode=PMODE,
            )

        def phase_scores(qb):
            """scores^T + exp, two k-tiles per PSUM bank-pair / ACTIVATE.
            Denominator: qb0 accumulates on the (idle there) DVE as
            incremental [128,1024] adds; qb1's scores window already has
            weighted-qb0 DVE work, so its denominator rides the PE as tiny
            ones-matmuls trailing one pair behind the exps."""
            q0 = qb * QB
            expT = st_pool.tile([128, KTILES * QB], FP8, tag=f"expT{qb}",
                                name=f"expT{qb}")
            state[qb] = {"expT": expT}
            if qb == 0:
                acc = st_pool.tile([128, 2 * QB], BF16, tag="acc0")
            else:
                state[qb]["ps_den"] = ps_one.tile(
                    [128, QB], dt.float32, tag="norm", name="ps_den1")
            for p8 in range(KTILES // 2):
                pair = ps_pair.tile([128, 2 * QB], dt.float32, tag="pair",
                                    name=f"ps_s_{qb}_{p8}")
                for half in range(2):
                    kt = 2 * p8 + half
                    for jo in range(HT // 2):
                        nc.tensor.matmul(
                            pair[:, half * QB:(half + 1) * QB],
                            KT3[:, 2 * jo: 2 * jo + 2, kt * 128:(kt + 1) * 128],
                            QT3[:, 2 * jo: 2 * jo + 2, q0: q0 + QB],
                            start=(jo == 0),
                            stop=(jo == HT // 2 - 1),
                            perf_mode=PMODE,
                        )
                sl = expT[:, p8 * 2 * QB:(p8 + 1) * 2 * QB]
                nc.scalar.activation(sl, pair[:], AF.Exp)
                if qb == 0:
                    if p8 == 0:
                        nc.vector.tensor_copy(acc[:], sl)
                    else:
                        nc.vector.tensor_add(acc[:], acc[:], sl)
                elif p8 >= 1:
                    den_mm(qb, p8 - 1)
            if qb == 0:
                sum_part = st_pool.tile([128, QB], dt.float32r, tag="sump0")
                nc.vector.tensor_add(sum_part[:], acc[:, :QB], acc[:, QB:])
                state[qb]["sum_part"] = sum_part

        def phase_norm_ln(qb):
            """Partition-reduce the rowsum (qb0: one ones-matmul on the DVE
            partial; qb1: close the PE den group), then 1/rowsum =
            exp(-ln(rowsum)) on ScalarE: no DVE reciprocal."""
            st = state[qb]
            if qb == 0:
                ps_den = ps_one.tile([128, QB], dt.float32, tag="norm",
                                     name="ps_den0")
                nc.tensor.matmul(ps_den[0:1, :], ones_col[:],
                                 st["sum_part"][:], start=True, stop=True)
            else:
                den_mm(qb, KTILES // 2 - 1)
                ps_den = st["ps_den"]
            logsum = st_pool.tile([1, QB], dt.float32r, tag="logsum",
                                  name=f"logsum{qb}")
            nc.scalar.activation(logsum[:], ps_den[0:1, :], AF.Ln)
            st["logsum"] = logsum

        def phase_norm_bcast(qb):
            """bcast64 = exp(ln64 - ln(rowsum)) = 64/rowsum: the x64 keeps
            the weighted residual in the same scale as the x64 Wo2 PSUM so
            the MLP output evacuates in ONE fused DVE op."""
            st = state[qb]
            ps_b = ps_one.tile([128, QB], dt.float32, tag="norm",
                               name=f"ps_b{qb}")
            nc.tensor.matmul(ps_b[:], ones_row[:], st["logsum"][:],
                             start=True, stop=True)
            bcast = st_pool.tile([128, QB], dt.float32, tag="bcast",
                                 name=f"bcast{qb}")
            nc.scalar.activation(bcast[:], ps_b[:], AF.Exp, scale=-1.0)
            st["bcast"] = bcast

        def phase_pv_half(qb, half):
            """PV for 3 h-tiles over all 16 k-tiles; norm chain of this
            q-block interleaves under half 0."""
            st = state[qb]
            rhs8 = st["expT"][:].rearrange("p (j t q) -> p j t q",
                                           j=KTILES // 2, t=2)
            ps_w = [ps_one.tile([128, QB], dt.float32, tag=f"pvw{i}",
                                name=f"pvw{i}_{qb}_{half}")
                    for i in range(3)]
            if half == 0:
                sched = [(jk, i) for jk in range(KTILES // 2) for i in range(3)]
            else:
                # skewed wavefront: bank i starts i waves late, so the first
                # matmul of each bank lands just after the previous half's
                # weighted-mul releases that bank (no lump wait on DVE)
                sched = [(w - i, i) for w in range(KTILES // 2 + 2)
                         for i in range(3) if 0 <= w - i < KTILES // 2]
            for n, (jk, i) in enumerate(sched):
                if half == 0 and (jk, i) == (1, 0):
                    phase_norm_ln(qb)
                if half == 0 and (jk, i) == (3, 0):
                    phase_norm_bcast(qb)
                nc.tensor.matmul(
                    ps_w[i][:],
                    vpair(jk, 3 * half + i),
                    rhs8[:, jk],
                    start=(jk == 0),
                    stop=(jk == KTILES // 2 - 1),
                    perf_mode=PMODE,
                )
            st[f"ps_w{half}"] = ps_w

        def phase_weighted_half(qb, half):
            """w = PV/rowsum + (value^T + bo2); bf16 residual + fp8 GEMM copy."""
            st = state[qb]
            ps_w = st[f"ps_w{half}"]
            if "wr" not in st:
                st["wr"] = st_pool.tile([128, HT * QB], BF16, tag=f"wr{qb}",
                                        name=f"wr{qb}")
                st["w8"] = st_pool.tile([128, HT * QB], FP8, tag=f"w8_{qb}",
                                        name=f"w8_{qb}")
            wr, w8 = st["wr"], st["w8"]
            for i in range(3):
                ht = 3 * half + i
                c0 = ht * QB
                tmp = out_pool.tile([128, QB], dt.float32, tag="wtmp",
                                    name=f"wtmp_{qb}_{ht}")
                nc.vector.tensor_mul(tmp[:], ps_w[i][:], st["bcast"][:])
                nc.vector.tensor_add(
                    wr[:, c0: c0 + QB], tmp[:],
                    vTb_in[:, qb * HT * QB + c0: qb * HT * QB + c0 + QB],
                )
                if (qb, half) == (0, 1):
                    # this cast's consumers sit behind scores1's exps on
                    # ScalarE, which is full there -- cast on DVE instead
                    nc.vector.tensor_copy(w8[:, c0: c0 + QB],
                                          wr[:, c0: c0 + QB])
                else:
                    nc.scalar.copy(w8[:, c0: c0 + QB], wr[:, c0: c0 + QB])

        def phase_mlp_h1(qb):
            st = state[qb]
            w8v = st["w8"][:].rearrange("p (t q) -> p t q", t=HT)
            wv = w3("wo1T")
            h1 = st_pool.tile([128, HT * QB], FP8, tag=f"h1_{qb}",
                              name=f"h1T{qb}")
            for otp in range(HT // 2):
                pair = ps_pair.tile([128, 2 * QB], dt.float32, tag="pair",
                                    name=f"ps_h1_{qb}_{otp}")
                for h in range(2):
                    ot = 2 * otp + h
                    for jo in range(HT // 2):
                        nc.tensor.matmul(
                            pair[:, h * QB:(h + 1) * QB],
                            wv[:, ot * HT + 2 * jo: ot * HT + 2 * jo + 2, :],
                            w8v[:, 2 * jo: 2 * jo + 2, :],
                            start=(jo == 0),
                            stop=(jo == HT // 2 - 1),
                            perf_mode=PMODE,
                        )
                for h in range(2):
                    ot = 2 * otp + h
                    nc.scalar.activation(
                        h1[:, ot * QB:(ot + 1) * QB],
                        pair[:, h * QB:(h + 1) * QB],
                        AF.Relu, bias=biases["bo1"][:, ot: ot + 1],
                        scale=1.0 / WO1_SCALE,
                    )
            st["h1"] = h1

        def phase_mlp_out(qb):
            """out = h1 @ Wo2^T + (w + bo2): PSUM + residual in one DVE add,
            straight to bf16 DMA staging."""
            st = state[qb]
            h1v = st["h1"][:].rearrange("p (t q) -> p t q", t=HT)
            wv = w3("wo2T")
            for otp in range(HT // 2):
                pair = ps_pair.tile([128, 2 * QB], dt.float32, tag="pair",
                                    name=f"ps_o_{qb}_{otp}")
                for h in range(2):
                    ot = 2 * otp + h
                    for jo in range(HT // 2):
                        nc.tensor.matmul(
                            pair[:, h * QB:(h + 1) * QB],
                            wv[:, ot * HT + 2 * jo: ot * HT + 2 * jo + 2, :],
                            h1v[:, 2 * jo: 2 * jo + 2, :],
                            start=(jo == 0),
                            stop=(jo == HT // 2 - 1),
                            perf_mode=PMODE,
                        )
                o_mid = out_pool.tile([128, 2 * QB], BF16, tag="o_mid",
                                      name=f"omid_{qb}_{otp}")
                o_sb = out_pool.tile([128, 2 * QB], BF16, tag="outT_blk",
                                     name=f"outT_{qb}_{otp}")
                c0 = (qb * HT + otp * 2) * QB
                # the very last block streams out in 512-wide chunks so the
                # act -> add -> DMA tail pipeline overlaps
                nchunk = 2 if (qb, otp) == (1, HT // 2 - 1) else 1
                for ch in range(nchunk):
                    w = 2 * QB // nchunk
                    sl = slice(ch * w, (ch + 1) * w)
                    nc.scalar.activation(o_mid[:, sl], pair[:, sl], AF.Identity,
                                         scale=1.0 / WO2_SCALE)
                    nc.vector.tensor_add(
                        o_sb[:, sl], o_mid[:, sl],
                        st["wr"][:, otp * 2 * QB + ch * w:
                                 otp * 2 * QB + (ch + 1) * w],
                    )
                    nc.sync.dma_start(
                        outT_ext[:, c0 + ch * w: c0 + (ch + 1) * w],
                        o_sb[:, sl],
                    )

        # software pipeline: DVE/ScalarE chains (norm, weighted, h1-acts) are
        # always covered by an independent PE phase emitted around them
        phase_scores(0)
        phase_pv_half(0, 0)
        phase_weighted_half(0, 0)
        phase_pv_half(0, 1)
        phase_weighted_half(0, 1)
        phase_scores(1)
        phase_mlp_h1(0)
        phase_pv_half(1, 0)
        phase_weighted_half(1, 0)
        phase_pv_half(1, 1)
        phase_weighted_half(1, 1)
        phase_mlp_out(0)
        phase_mlp_h1(1)
        phase_mlp_out(1)


# ---- host-side shard packing ----

def _tile_rows(a):
    """[T*128, N] -> [128, T*N]: partition-tiled T-layout, contiguous DMA."""
    t = a.shape[0] // 128
    return a.reshape(t, 128, a.shape[1]).transpose(1, 0, 2).reshape(128, -1)


def _tile_weight(w):
    """W^T [768h, 768o] -> [128, (ot, ht, 128)]: o-major packed lhsT tiles."""
    x = w.reshape(HT, 128, HT, 128)          # [ht, p, ot, o128]
    return x.transpose(1, 2, 0, 3).reshape(128, -1)


def _tile_rows_blocked(a, qb):
    """[768, NB*qb] -> [128, NB*(6*qb)]: per-block ht-major packing."""
    nb = a.shape[1] // qb
    x = a.reshape(HT, 128, nb, qb).transpose(1, 2, 0, 3)
    return x.reshape(128, -1)


def shard_inputs(query, key, value, Wq, bq, Wk, bk, Wo1, bo1, Wo2, bo2):
    """Full inputs -> per-core in_maps (host packing, fp8 cast, scale folds)."""
    scale = np.float32(1.0 / np.sqrt(np.float32(H)))

    def c8(x):
        return np.ascontiguousarray(
            np.clip(np.asarray(x, np.float32), -240, 240).astype(NP_FP8))

    def cb(x):
        return np.ascontiguousarray(np.asarray(x, np.float32).astype(NP_BF16))

    def cf(x):
        return np.ascontiguousarray(x.astype(np.float32))

    bo1p = bo1 - Wo1 @ bo2           # corrects for the +bo2 folded into w'
    shared = {
        "wqT": c8(_tile_weight(Wq.T * (scale * WQ_SCALE))),
        "wkT": c8(_tile_weight(Wk.T * WK_SCALE)),
        "wo1T": c8(_tile_weight(Wo1.T * WO1_SCALE)),
        "wo2T": c8(_tile_weight(Wo2.T * WO2_SCALE)),
        "biases": cf(np.concatenate([
            (bq * scale).reshape(HT, 128).T, bk.reshape(HT, 128).T,
            np.asarray(bo1p).reshape(HT, 128).T], axis=1)),
    }
    in_maps = []
    for core in range(N_CORES):
        b, half = divmod(core, 2)
        r0 = half * QCHUNK
        vTb = np.asarray(value[b]).T + np.asarray(bo2)[:, None]
        in_maps.append({
            "qT": c8(_tile_rows_blocked(query[b].T[:, r0: r0 + QCHUNK], QB)),
            "kT": c8(_tile_rows_blocked(key[b].T, QB)),
            "v": c8(_tile_rows(np.asarray(value[b]))),
            "vTb": cb(_tile_rows_blocked(vTb[:, r0: r0 + QCHUNK], QB)),
            **shared,
        })
    return in_maps


def gather_outputs(results):
    """Per-core outT [128, NQB*HT*QB] bf16 -> full [B, S, H] fp32."""
    out = np.empty((B, S, H), dtype=np.float32)
    for core in range(N_CORES):
        b, half = divmod(core, 2)
        r0 = half * QCHUNK
        buf = results[core]["outT"].reshape(128, NQB, HT, QB)
        # out[q0+qb*QB+n, ot*128+p] = buf[p, qb, ot, n]
        out[b, r0: r0 + QCHUNK] = (
            buf.transpose(1, 3, 2, 0).reshape(QCHUNK, H).astype(np.float32)
        )
    return out


def run(inputs, trace=False):
    nc = build_kernel()
    in_maps = shard_inputs(**{k: np.asarray(v) for k, v in inputs.items()})
    res = run_bass_kernel_spmd(nc, in_maps, list(range(N_CORES)), trace=trace)
    return gather_outputs(res.results), res


def _split_multi_waits(nc):
    """Workaround for this container's walrus rejecting instructions that
    carry more than one semaphore wait ("Too many sync wait commands"):
    hoist N-1 waits onto fresh single-wait same-engine InstNoOp instructions
    inserted immediately before the instruction. Engine streams execute the
    block's per-engine subsequence in order, so blocking on the nops first is
    semantically identical to one multi-wait instruction."""
    for f in nc.m.functions:
        for bb in f.blocks:
            insts = list(bb.instructions)
            out = []
            changed = False
            for inst in insts:
                si = inst.sync_info
                waits = list(si.on_wait) if si is not None and si.on_wait else []
                if len(waits) > 1:
                    changed = True
                    for w in waits[:-1]:
                        nop = mybir.InstNoOp(
                            name=nc.get_next_instruction_name(), ins=[], outs=[]
                        )
                        nop.engine = inst.engine
                        nop.sync_info = mybir.SyncInfo(on_wait=[w], on_update=[])
                        out.append(nop)
                    si.on_wait = waits[-1:]
                    inst.sync_info = si
                out.append(inst)
            if changed:
                bb.instructions = out


def kernel(**inputs):
    """Entry point: full (unsharded) numpy inputs -> full [B, S, H] output."""
    out, _ = run(inputs, trace=False)
    return out


# revision 47
# speedup vs baseline: 1.2431x; 1.0087x over previous
"""Distributed single-head attention + MLP block for 8 TRN2 NeuronCores.

Reference computation (per batch b):
  Q = query @ Wq^T + bq ; K = key @ Wk^T + bk
  scores = Q @ K^T / sqrt(H) ; attn = softmax(scores)
  weighted = attn @ value + value
  h1 = relu(weighted @ Wo1^T + bo1)
  out = h1 @ Wo2^T + bo2 + weighted

Sharding: B=4 batches x 2 query-row halves = 8 shards. Each core gets its
1024 query rows plus the full 2048 keys/values of its batch; attention is
dense (non-causal) so no inter-core communication is needed.

Layout strategy: everything on-device lives feature-on-partitions
("T-layout", X^T[f, tok]) so all matmul contractions line up with zero
on-device transposes, and the host pre-packs every shard into the exact
[128, free] SBUF tiling the kernel consumes. All five GEMMs run in fp8
DoubleRow (2 contraction rows/cycle); fp8 weight tensors are scaled up
x64/x256 on the host to escape the e4m3 subnormal range and un-scaled for
free via the ScalarE activation's scale argument. The softmax needs no
max-subtraction: scores have std ~1/3 by construction.

Residual/bias algebra: the host ships vTb = value^T + bo2, so the kernel's
"weighted + bo2" residual costs nothing; feeding the MLP with w' = w + bo2
is corrected by bo1' = bo1 - Wo1 @ bo2 (exact), which lets the second MLP
GEMM skip ScalarE entirely (one DVE add straight out of PSUM).

PSUM is laid out as 2x rotating [128,1024] bank-pairs (scores/projection/
MLP accumulators, evacuated by ONE wide ScalarE activation each) + 3
single banks for the PV accumulation (processed in two ht-halves) + 1
norm/warmup bank. The softmax denominator is a 4-op contiguous DVE fold
tree over the [128, 16*512] exp tile, then the usual ones-matmul rowsum +
reciprocal + PE-broadcast. A dozen throwaway matmuls run during the
initial DMA wait to trip the PE HAM clock-gate to 2.4 GHz before the real
GEMM stream starts.
"""

import contextlib

import numpy as np
import ml_dtypes

import concourse.bass as bass
import concourse.mybir as mybir
import concourse.tile as tile
from concourse.bass_utils import run_bass_kernel_spmd

dt = mybir.dt
AF = mybir.ActivationFunctionType

H = 768          # model dim
B = 4            # batch
S = 2048         # sequence length
N_CORES = 8
QCHUNK = S * B // N_CORES        # 1024 query rows per core
HT = H // 128                    # 6 feature partition-tiles
KTILES = S // 128                # 16 key partition-tiles
QB = 512                         # q-block width (= PSUM bank, fp32)
NQB = QCHUNK // QB               # 2 q-blocks per core

FP8 = dt.float8e4
NP_FP8 = dt.np(FP8)
BF16 = dt.bfloat16
NP_BF16 = ml_dtypes.bfloat16
PMODE = mybir.MatmulPerfMode.DoubleRow

WQ_SCALE = 256.0                 # host premultiplier on Wq (incl 1/sqrt(H))
WK_SCALE = 64.0                  # host premultiplier on Wk
WO1_SCALE = 64.0                 # host premultiplier on Wo1
WO2_SCALE = 64.0                 # host premultiplier on Wo2


def build_kernel():
    nc = bass.Bass()

    qT_ext = nc.declare_dram_parameter("qT", [128, HT * QCHUNK], FP8, isOutput=False)
    kT_ext = nc.declare_dram_parameter("kT", [128, HT * S], FP8, isOutput=False)
    v_ext = nc.declare_dram_parameter("v", [128, KTILES * H], FP8, isOutput=False)
    vTb_ext = nc.declare_dram_parameter("vTb", [128, HT * QCHUNK], BF16,
                                        isOutput=False)
    w_ext = {
        name: nc.declare_dram_parameter(name, [128, HT * H], FP8, isOutput=False)
        for name in ("wqT", "wkT", "wo1T", "wo2T")
    }
    b_ext = nc.declare_dram_parameter("biases", [128, 3 * HT], dt.float32,
                                      isOutput=False)
    outT_ext = nc.declare_dram_parameter(
        "outT", [128, HT * QCHUNK], BF16, isOutput=True
    )

    with tile.TileContext(nc) as tc, nc.allow_low_precision(
        reason="fp8 matmul path is intentional; rel-err budget is 2e-2"
    ):
        _body(nc, tc, qT_ext, kT_ext, v_ext, vTb_ext, w_ext, b_ext, outT_ext)

    _split_multi_waits(nc)
    return nc


def _body(nc, tc, qT_ext, kT_ext, v_ext, vTb_ext, w_ext, b_ext, outT_ext):
    with contextlib.ExitStack() as ctx:
        const_pool = ctx.enter_context(tc.tile_pool(name="const", bufs=1))
        w_pool = ctx.enter_context(tc.tile_pool(name="w", bufs=1))
        act_pool = ctx.enter_context(tc.tile_pool(name="act", bufs=1))
        st_pool = ctx.enter_context(tc.tile_pool(name="st", bufs=1))
        out_pool = ctx.enter_context(tc.tile_pool(name="outs", bufs=3))
        # PSUM: 2 x [128,1024] rotating bank-pairs + 3 PV banks + 1 norm bank.
        ps_pair = ctx.enter_context(tc.tile_pool(name="ps_pair", bufs=2,
                                                 space="PSUM"))
        ps_one = ctx.enter_context(tc.tile_pool(name="ps_one", bufs=1,
                                                space="PSUM"))

        # ---- constants + PE warm-up (no DMA dependency: runs during the
        # input DMA head and trips the HAM clock gate to full rate) ----
        wu = const_pool.tile([128, 256], BF16, tag="warmup")
        nc.vector.memset(wu[:], 0.002)
        ones_f32 = const_pool.tile([128, 128], dt.float32, tag="ones_f32")
        nc.vector.memset(ones_f32[:], 1.0)
        ones_row = const_pool.tile([1, 128], dt.float32r, tag="ones_row")
        nc.vector.tensor_copy(ones_row[:], ones_f32[0:1, :])
        ones_col = const_pool.tile([128, 1], dt.float32r, tag="ones_col")
        nc.vector.tensor_copy(ones_col[:], ones_f32[:, 0:1])
        # fp8 ones pair for the qb1 denominator matmul: DR lhsT needs the
        # k-tile step to be a multiple of 16 bytes, so cols 0 and 16 of a
        # 32-wide tile are the two "rows" the AP actually reads.
        ones8 = const_pool.tile([128, 32], FP8, tag="ones8")
        nc.vector.memset(ones8[:], 1.0)
        ones8v = ones8[:].rearrange("p (t m) -> p t m", t=2)[:, :, 0:1]
        ln64 = const_pool.tile([128, 1], dt.float32, tag="ln64")
        nc.vector.memset(ln64[:], float(np.log(WO2_SCALE)))

        # ---- input DMAs in first-use order. The first-matmul chain (kT
        # block A, wkT chunk 0, biases, rest of wkT, kT block B) rides the
        # sync queue; ScalarE's queue opens with a dummy Ln activation that
        # pre-pays the ~2.7us ACT table load while those DMAs fly (Ln selects
        # natural_log_exp_and_others, which also holds Exp/Relu/Identity/
        # Copy, so this is the only table load in the kernel). ----
        # wkT chunks ride the scalar HWDGE queue (parallel with kT on sync),
        # issued BEFORE the dummy Ln act so the table load doesn't delay them
        w_sb = {
            name: w_pool.tile([128, HT * H], FP8, tag=name, name=f"w_{name}")
            for name in ("wqT", "wkT", "wo1T", "wo2T")
        }
        for c0, c1 in ((0, 1), (1, 2), (2, 3), (3, HT)):
            nc.scalar.dma_start(w_sb["wkT"][:, c0 * HT * 128: c1 * HT * 128],
                                w_ext["wkT"][:, c0 * HT * 128: c1 * HT * 128])
        actwarm = const_pool.tile([1, 2], dt.float32, tag="actwarm")
        nc.scalar.activation(actwarm[:], ones_f32[0:1, 0:2], AF.Ln)

        # kT block A lands in contraction order (ht tiles 0-3 of both
        # sub-blocks first) so the first projection matmuls start sooner
        kT_in = act_pool.tile([128, HT * S], FP8, tag="kT_in")
        C2 = 4 * 512
        for a, b in ((0, C2), (3072, 3072 + C2), (C2, 3072), (3072 + C2, 6144)):
            nc.sync.dma_start(kT_in[:, a:b], kT_ext[:, a:b])
        bias_sb = const_pool.tile([128, 3 * HT], dt.float32, tag="biases")
        nc.sync.dma_start(bias_sb[:], b_ext[:])
        biases = {name: bias_sb[:, i * HT:(i + 1) * HT]
                  for i, name in enumerate(("bq", "bk", "bo1"))}
        nc.sync.dma_start(kT_in[:, HT * QB * 2:], kT_ext[:, HT * QB * 2:])
        nc.scalar.dma_start(w_sb["wqT"][:], w_ext["wqT"][:])
        qT_in = act_pool.tile([128, HT * QCHUNK], FP8, tag="qT_in")
        nc.sync.dma_start(qT_in[:], qT_ext[:])
        v_sb = act_pool.tile([128, KTILES * H], FP8, tag="v_in")
        nc.sync.dma_start(v_sb[:], v_ext[:])
        nc.sync.dma_start(w_sb["wo1T"][:], w_ext["wo1T"][:])
        nc.sync.dma_start(w_sb["wo2T"][:], w_ext["wo2T"][:])
        vTb_in = act_pool.tile([128, HT * QCHUNK], BF16, tag="vTb_in")
        nc.sync.dma_start(vTb_in[:], vTb_ext[:])

        # warm-up matmuls (~1.5us of junk PE work into the norm bank)
        for i in range(7):
            ps_wu = ps_one.tile([128, QB], dt.float32, tag="norm",
                                name=f"wu{i}")
            nc.tensor.matmul(ps_wu[:, :256], wu[:, :128], wu[:],
                             start=True, stop=True)

        def w3(name):
            return w_sb[name][:].rearrange("p (o t m) -> p (o t) m", o=HT, t=HT)

        # ---- K/Q projections: jo-contraction into [128,1024] bank-pairs,
        # one wide ScalarE activation per (ot, block-pair) ----
        def project(wname, bias, scale, x_in, nblocks, out_sb, out_col, tag,
                    head_split=False):
            wv = w3(wname)
            xv = x_in[:].rearrange("p (n t q) -> p n t q", n=nblocks, t=HT)
            head_banks = {(0, 0): "pvw0", (0, 1): "pvw1",
                          (1, 0): "pvw2", (1, 1): "norm"}
            deferred = []
            for nb2 in range(nblocks // 2):
                for ot in range(HT):
                    # the very first evacuations ride 4 idle single banks so
                    # the act-latency ramp doesn't stall the bank-pair pool
                    split = head_split and nb2 == 0 and ot < 2
                    if split:
                        tiles = [ps_one.tile([128, QB], dt.float32,
                                             tag=head_banks[(ot, h)],
                                             name=f"ps_{tag}h_{ot}_{h}")
                                 for h in range(2)]
                        tgt = lambda h: tiles[h][:]
                    else:
                        pair = ps_pair.tile([128, 2 * QB], dt.float32,
                                            tag="pair",
                                            name=f"ps_{tag}_{nb2}_{ot}")
                        tgt = lambda h: pair[:, h * QB:(h + 1) * QB]
                    for jo in range(HT // 2):
                        for h in range(2):
                            nc.tensor.matmul(
                                tgt(h),
                                wv[:, ot * HT + 2 * jo: ot * HT + 2 * jo + 2, :],
                                xv[:, 2 * nb2 + h, 2 * jo: 2 * jo + 2, :],
                                start=(jo == 0),
                                stop=(jo == HT // 2 - 1),
                                perf_mode=PMODE,
                            )
                    c0 = out_col(ot, nb2)
                    if split:
                        # defer these evacuations: their banks aren't needed
                        # until PV, and emitting them later keeps the first
                        # NEEDED act (ot2's, gating ot4) at the queue front
                        deferred.append((c0, tiles, bias[:, ot: ot + 1]))
                    else:
                        nc.scalar.activation(
                            out_sb[:, c0: c0 + 2 * QB], pair[:], AF.Identity,
                            bias=bias[:, ot: ot + 1], scale=scale,
                        )
                        if deferred and ot == 3:
                            for d0, dtiles, db in deferred:
                                for h in range(2):
                                    nc.scalar.activation(
                                        out_sb[:, d0 + h * QB:
                                               d0 + (h + 1) * QB],
                                        dtiles[h][:], AF.Identity,
                                        bias=db, scale=scale,
                                    )
                            deferred = []

        KT = act_pool.tile([128, HT * S], FP8, tag="KT", name="KT_full")
        project("wkT", biases["bk"], 1.0 / WK_SCALE, kT_in, 4, KT,
                lambda ot, nb2: ot * S + nb2 * 2 * QB, "KT", head_split=True)
        QT = act_pool.tile([128, HT * QCHUNK], FP8, tag="QT", name="proj_QT")
        project("wqT", biases["bq"], 1.0 / WQ_SCALE, qT_in, 2, QT,
                lambda ot, nb2: ot * QCHUNK + nb2 * 2 * QB, "QT")
        KT3 = KT[:].rearrange("p (t k) -> p t k", t=HT)
        QT3 = QT[:].rearrange("p (t q) -> p t q", t=HT)

        def vpair(jk, ht):
            """lhsT [128, 2, 128]: k-tile pair (2jk, 2jk+1), h-tile ht."""
            return (v_sb[:].rearrange("p (t h) -> p t h", t=KTILES)
                    [:, 2 * jk: 2 * jk + 2, ht * 128:(ht + 1) * 128])

        # ---- attention + MLP, software-pipelined across q-blocks ----
        state = {}

        def den_mm(qb, p8):
            """qb1 path: accumulate exp-pair p8 into the [1,512] rowsum via
            a ones matmul (contraction over 128 partitions x 2 k-tiles)."""
            st = state[qb]
            rhs8 = st["expT"][:].rearrange("p (j t q) -> p j t q",
                                           j=KTILES // 2, t=2)
            nc.tensor.matmul(
                st["ps_den"][0:1, :], ones8v, rhs8[:, p8],
                start=(p8 == 0), stop=(p8 == KTILES // 2 - 1),
                perf_mode=PMODE,
            )

        def phase_scores(qb):
            """scores^T + exp, two k-tiles per PSUM bank-pair / ACTIVATE.
            Denominator: qb0 accumulates on the (idle there) DVE as
            incremental [128,1024] adds; qb1's scores window already has
            weighted-qb0 DVE work, so its denominator rides the PE as tiny
            ones-matmuls trailing one pair behind the exps."""
            q0 = qb * QB
            expT = st_pool.tile([128, KTILES * QB], FP8, tag=f"expT{qb}",
                                name=f"expT{qb}")
            state[qb] = {"expT": expT}
            if qb == 0:
                acc = st_pool.tile([128, 2 * QB], BF16, tag="acc0")
            else:
                state[qb]["ps_den"] = ps_one.tile(
                    [128, QB], dt.float32, tag="norm", name="ps_den1")
            for p8 in range(KTILES // 2):
                pair = ps_pair.tile([128, 2 * QB], dt.float32, tag="pair",
                                    name=f"ps_s_{qb}_{p8}")
                for half in range(2):
                    kt = 2 * p8 + half
                    for jo in range(HT // 2):
                        nc.tensor.matmul(
                            pair[:, half * QB:(half + 1) * QB],
                            KT3[:, 2 * jo: 2 * jo + 2, kt * 128:(kt + 1) * 128],
                            QT3[:, 2 * jo: 2 * jo + 2, q0: q0 + QB],
                            start=(jo == 0),
                            stop=(jo == HT // 2 - 1),
                            perf_mode=PMODE,
                        )
                sl = expT[:, p8 * 2 * QB:(p8 + 1) * 2 * QB]
                nc.scalar.activation(sl, pair[:], AF.Exp)
                if qb == 0:
                    if p8 == 0:
                        nc.vector.tensor_copy(acc[:], sl)
                    else:
                        nc.vector.tensor_add(acc[:], acc[:], sl)
                elif p8 >= 1:
                    den_mm(qb, p8 - 1)
            if qb == 0:
                sum_part = st_pool.tile([128, QB], dt.float32r, tag="sump0")
                nc.vector.tensor_add(sum_part[:], acc[:, :QB], acc[:, QB:])
                state[qb]["sum_part"] = sum_part

        def phase_norm_ln(qb):
            """Partition-reduce the rowsum (qb0: one ones-matmul on the DVE
            partial; qb1: close the PE den group), then 1/rowsum =
            exp(-ln(rowsum)) on ScalarE: no DVE reciprocal."""
            st = state[qb]
            if qb == 0:
                ps_den = ps_one.tile([128, QB], dt.float32, tag="norm",
                                     name="ps_den0")
                nc.tensor.matmul(ps_den[0:1, :], ones_col[:],
                                 st["sum_part"][:], start=True, stop=True)
            else:
                den_mm(qb, KTILES // 2 - 1)
                ps_den = st["ps_den"]
            logsum = st_pool.tile([1, QB], dt.float32r, tag="logsum",
                                  name=f"logsum{qb}")
            nc.scalar.activation(logsum[:], ps_den[0:1, :], AF.Ln)
            st["logsum"] = logsum

        def phase_norm_bcast(qb):
            """bcast64 = exp(ln64 - ln(rowsum)) = 64/rowsum: the x64 keeps
            the weighted residual in the same scale as the x64 Wo2 PSUM so
            the MLP output evacuates in ONE fused DVE op."""
            st = state[qb]
            ps_b = ps_one.tile([128, QB], dt.float32, tag="norm",
                               name=f"ps_b{qb}")
            nc.tensor.matmul(ps_b[:], ones_row[:], st["logsum"][:],
                             start=True, stop=True)
            bcast = st_pool.tile([128, QB], dt.float32, tag="bcast",
                                 name=f"bcast{qb}")
            nc.scalar.activation(bcast[:], ps_b[:], AF.Exp, scale=-1.0)
            st["bcast"] = bcast

        def phase_pv_half(qb, half):
            """PV for 3 h-tiles over all 16 k-tiles; norm chain of this
            q-block interleaves under half 0."""
            st = state[qb]
            rhs8 = st["expT"][:].rearrange("p (j t q) -> p j t q",
                                           j=KTILES // 2, t=2)
            ps_w = [ps_one.tile([128, QB], dt.float32, tag=f"pvw{i}",
                                name=f"pvw{i}_{qb}_{half}")
                    for i in range(3)]
            if half == 0:
                sched = [(jk, i) for jk in range(KTILES // 2) for i in range(3)]
            else:
                # skewed wavefront: bank i starts i waves late, so the first
                # matmul of each bank lands just after the previous half's
                # weighted-mul releases that bank (no lump wait on DVE)
                sched = [(w - i, i) for w in range(KTILES // 2 + 2)
                         for i in range(3) if 0 <= w - i < KTILES // 2]
            for n, (jk, i) in enumerate(sched):
                if half == 0 and (jk, i) == (1, 0):
                    phase_norm_ln(qb)
                if half == 0 and (jk, i) == (3, 0):
                    phase_norm_bcast(qb)
                nc.tensor.matmul(
                    ps_w[i][:],
                    vpair(jk, 3 * half + i),
                    rhs8[:, jk],
                    start=(jk == 0),
                    stop=(jk == KTILES // 2 - 1),
                    perf_mode=PMODE,
                )
            st[f"ps_w{half}"] = ps_w

        def phase_weighted_half(qb, half):
            """w = PV/rowsum + (value^T + bo2); bf16 residual + fp8 GEMM copy."""
            st = state[qb]
            ps_w = st[f"ps_w{half}"]
            if "wr" not in st:
                st["wr"] = st_pool.tile([128, HT * QB], BF16, tag=f"wr{qb}",
                                        name=f"wr{qb}")
                st["w8"] = st_pool.tile([128, HT * QB], FP8, tag=f"w8_{qb}",
                                        name=f"w8_{qb}")
            wr, w8 = st["wr"], st["w8"]
            for i in range(3):
                ht = 3 * half + i
                c0 = ht * QB
                tmp = out_pool.tile([128, QB], dt.float32, tag="wtmp",
                                    name=f"wtmp_{qb}_{ht}")
                nc.vector.tensor_mul(tmp[:], ps_w[i][:], st["bcast"][:])
                nc.vector.tensor_add(
                    wr[:, c0: c0 + QB], tmp[:],
                    vTb_in[:, qb * HT * QB + c0: qb * HT * QB + c0 + QB],
                )
                if half == 1:
                    # half-1 casts' queue slots on ScalarE would sit in
                    # front of (and so gate) the next phase's activations
                    # under the coarse per-engine semaphores -- DVE instead
                    nc.vector.tensor_copy(w8[:, c0: c0 + QB],
                                          wr[:, c0: c0 + QB])
                else:
                    nc.scalar.copy(w8[:, c0: c0 + QB], wr[:, c0: c0 + QB])

        def phase_mlp_h1(qb):
            st = state[qb]
            w8v = st["w8"][:].rearrange("p (t q) -> p t q", t=HT)
            wv = w3("wo1T")
            h1 = st_pool.tile([128, HT * QB], FP8, tag=f"h1_{qb}",
                              name=f"h1T{qb}")
            for otp in range(HT // 2):
                pair = ps_pair.tile([128, 2 * QB], dt.float32, tag="pair",
                                    name=f"ps_h1_{qb}_{otp}")
                for h in range(2):
                    ot = 2 * otp + h
                    for jo in range(HT // 2):
                        nc.tensor.matmul(
                            pair[:, h * QB:(h + 1) * QB],
                            wv[:, ot * HT + 2 * jo: ot * HT + 2 * jo + 2, :],
                            w8v[:, 2 * jo: 2 * jo + 2, :],
                            start=(jo == 0),
                            stop=(jo == HT // 2 - 1),
                            perf_mode=PMODE,
                        )
                for h in range(2):
                    ot = 2 * otp + h
                    nc.scalar.activation(
                        h1[:, ot * QB:(ot + 1) * QB],
                        pair[:, h * QB:(h + 1) * QB],
                        AF.Relu, bias=biases["bo1"][:, ot: ot + 1],
                        scale=1.0 / WO1_SCALE,
                    )
            st["h1"] = h1

        def phase_mlp_out(qb):
            """out = h1 @ Wo2^T + (w + bo2): PSUM + residual in one DVE add,
            straight to bf16 DMA staging."""
            st = state[qb]
            h1v = st["h1"][:].rearrange("p (t q) -> p t q", t=HT)
            wv = w3("wo2T")
            for otp in range(HT // 2):
                pair = ps_pair.tile([128, 2 * QB], dt.float32, tag="pair",
                                    name=f"ps_o_{qb}_{otp}")
                for h in range(2):
                    ot = 2 * otp + h
                    for jo in range(HT // 2):
                        nc.tensor.matmul(
                            pair[:, h * QB:(h + 1) * QB],
                            wv[:, ot * HT + 2 * jo: ot * HT + 2 * jo + 2, :],
                            h1v[:, 2 * jo: 2 * jo + 2, :],
                            start=(jo == 0),
                            stop=(jo == HT // 2 - 1),
                            perf_mode=PMODE,
                        )
                o_mid = out_pool.tile([128, 2 * QB], BF16, tag="o_mid",
                                      name=f"omid_{qb}_{otp}")
                o_sb = out_pool.tile([128, 2 * QB], BF16, tag="outT_blk",
                                     name=f"outT_{qb}_{otp}")
                c0 = (qb * HT + otp * 2) * QB
                # the very last block streams out in 512-wide chunks so the
                # act -> add -> DMA tail pipeline overlaps
                nchunk = 2 if (qb, otp) == (1, HT // 2 - 1) else 1
                for ch in range(nchunk):
                    w = 2 * QB // nchunk
                    sl = slice(ch * w, (ch + 1) * w)
                    nc.scalar.activation(o_mid[:, sl], pair[:, sl], AF.Identity,
                                         scale=1.0 / WO2_SCALE)
                    nc.vector.tensor_add(
                        o_sb[:, sl], o_mid[:, sl],
                        st["wr"][:, otp * 2 * QB + ch * w:
                                 otp * 2 * QB + (ch + 1) * w],
                    )
                    nc.sync.dma_start(
                        outT_ext[:, c0 + ch * w: c0 + (ch + 1) * w],
                        o_sb[:, sl],
                    )

        # software pipeline: DVE/ScalarE chains (norm, weighted, h1-acts) are
        # always covered by an independent PE phase emitted around them
        phase_scores(0)
        phase_pv_half(0, 0)
        phase_weighted_half(0, 0)
        phase_pv_half(0, 1)
        phase_weighted_half(0, 1)
        phase_scores(1)
        phase_mlp_h1(0)
        phase_pv_half(1, 0)
        phase_weighted_half(1, 0)
        phase_pv_half(1, 1)
        phase_weighted_half(1, 1)
        phase_mlp_out(0)
        phase_mlp_h1(1)
        phase_mlp_out(1)


# ---- host-side shard packing ----

def _tile_rows(a):
    """[T*128, N] -> [128, T*N]: partition-tiled T-layout, contiguous DMA."""
    t = a.shape[0] // 128
    return a.reshape(t, 128, a.shape[1]).transpose(1, 0, 2).reshape(128, -1)


def _tile_weight(w):
    """W^T [768h, 768o] -> [128, (ot, ht, 128)]: o-major packed lhsT tiles."""
    x = w.reshape(HT, 128, HT, 128)          # [ht, p, ot, o128]
    return x.transpose(1, 2, 0, 3).reshape(128, -1)


def _tile_rows_blocked(a, qb):
    """[768, NB*qb] -> [128, NB*(6*qb)]: per-block ht-major packing."""
    nb = a.shape[1] // qb
    x = a.reshape(HT, 128, nb, qb).transpose(1, 2, 0, 3)
    return x.reshape(128, -1)


def shard_inputs(query, key, value, Wq, bq, Wk, bk, Wo1, bo1, Wo2, bo2):
    """Full inputs -> per-core in_maps (host packing, fp8 cast, scale folds)."""
    scale = np.float32(1.0 / np.sqrt(np.float32(H)))

    def c8(x):
        return np.ascontiguousarray(
            np.clip(np.asarray(x, np.float32), -240, 240).astype(NP_FP8))

    def cb(x):
        return np.ascontiguousarray(np.asarray(x, np.float32).astype(NP_BF16))

    def cf(x):
        return np.ascontiguousarray(x.astype(np.float32))

    bo1p = bo1 - Wo1 @ bo2           # corrects for the +bo2 folded into w'
    shared = {
        "wqT": c8(_tile_weight(Wq.T * (scale * WQ_SCALE))),
        "wkT": c8(_tile_weight(Wk.T * WK_SCALE)),
        "wo1T": c8(_tile_weight(Wo1.T * WO1_SCALE)),
        "wo2T": c8(_tile_weight(Wo2.T * WO2_SCALE)),
        "biases": cf(np.concatenate([
            (bq * scale).reshape(HT, 128).T, bk.reshape(HT, 128).T,
            np.asarray(bo1p).reshape(HT, 128).T], axis=1)),
    }
    in_maps = []
    for core in range(N_CORES):
        b, half = divmod(core, 2)
        r0 = half * QCHUNK
        vTb = np.asarray(value[b]).T + np.asarray(bo2)[:, None]
        in_maps.append({
            "qT": c8(_tile_rows_blocked(query[b].T[:, r0: r0 + QCHUNK], QB)),
            "kT": c8(_tile_rows_blocked(key[b].T, QB)),
            "v": c8(_tile_rows(np.asarray(value[b]))),
            "vTb": cb(_tile_rows_blocked(vTb[:, r0: r0 + QCHUNK], QB)),
            **shared,
        })
    return in_maps


def gather_outputs(results):
    """Per-core outT [128, NQB*HT*QB] bf16 -> full [B, S, H] fp32."""
    out = np.empty((B, S, H), dtype=np.float32)
    for core in range(N_CORES):
        b, half = divmod(core, 2)
        r0 = half * QCHUNK
        buf = results[core]["outT"].reshape(128, NQB, HT, QB)
        # out[q0+qb*QB+n, ot*128+p] = buf[p, qb, ot, n]
        out[b, r0: r0 + QCHUNK] = (
            buf.transpose(1, 3, 2, 0).reshape(QCHUNK, H).astype(np.float32)
        )
    return out


def run(inputs, trace=False):
    nc = build_kernel()
    in_maps = shard_inputs(**{k: np.asarray(v) for k, v in inputs.items()})
    res = run_bass_kernel_spmd(nc, in_maps, list(range(N_CORES)), trace=trace)
    return gather_outputs(res.results), res


def _split_multi_waits(nc):
    """Workaround for this container's walrus rejecting instructions that
    carry more than one semaphore wait ("Too many sync wait commands"):
    hoist N-1 waits onto fresh single-wait same-engine InstNoOp instructions
    inserted immediately before the instruction. Engine streams execute the
    block's per-engine subsequence in order, so blocking on the nops first is
    semantically identical to one multi-wait instruction."""
    for f in nc.m.functions:
        for bb in f.blocks:
            insts = list(bb.instructions)
            out = []
            changed = False
            for inst in insts:
                si = inst.sync_info
                waits = list(si.on_wait) if si is not None and si.on_wait else []
                if len(waits) > 1:
                    changed = True
                    for w in waits[:-1]:
                        nop = mybir.InstNoOp(
                            name=nc.get_next_instruction_name(), ins=[], outs=[]
                        )
                        nop.engine = inst.engine
                        nop.sync_info = mybir.SyncInfo(on_wait=[w], on_update=[])
                        out.append(nop)
                    si.on_wait = waits[-1:]
                    inst.sync_info = si
                out.append(inst)
            if changed:
                bb.instructions = out


def kernel(**inputs):
    """Entry point: full (unsharded) numpy inputs -> full [B, S, H] output."""
    out, _ = run(inputs, trace=False)
    return out


# revision 49
# speedup vs baseline: 1.2635x; 1.0164x over previous
"""Distributed single-head attention + MLP block for 8 TRN2 NeuronCores.

Reference computation (per batch b):
  Q = query @ Wq^T + bq ; K = key @ Wk^T + bk
  scores = Q @ K^T / sqrt(H) ; attn = softmax(scores)
  weighted = attn @ value + value
  h1 = relu(weighted @ Wo1^T + bo1)
  out = h1 @ Wo2^T + bo2 + weighted

Sharding: B=4 batches x 2 query-row halves = 8 shards. Each core gets its
1024 query rows plus the full 2048 keys/values of its batch; attention is
dense (non-causal) so no inter-core communication is needed.

Layout strategy: everything on-device lives feature-on-partitions
("T-layout", X^T[f, tok]) so all matmul contractions line up with zero
on-device transposes, and the host pre-packs every shard into the exact
[128, free] SBUF tiling the kernel consumes. All five GEMMs run in fp8
DoubleRow (2 contraction rows/cycle); fp8 weight tensors are scaled up
x64/x256 on the host to escape the e4m3 subnormal range and un-scaled for
free via the ScalarE activation's scale argument. The softmax needs no
max-subtraction: scores have std ~1/3 by construction.

Residual/bias algebra: the host ships vTb = value^T + bo2, so the kernel's
"weighted + bo2" residual costs nothing; feeding the MLP with w' = w + bo2
is corrected by bo1' = bo1 - Wo1 @ bo2 (exact), which lets the second MLP
GEMM skip ScalarE entirely (one DVE add straight out of PSUM).

PSUM is laid out as 2x rotating [128,1024] bank-pairs (scores/projection/
MLP accumulators, evacuated by ONE wide ScalarE activation each) + 3
single banks for the PV accumulation (processed in two ht-halves) + 1
norm/warmup bank. The softmax denominator is a 4-op contiguous DVE fold
tree over the [128, 16*512] exp tile, then the usual ones-matmul rowsum +
reciprocal + PE-broadcast. A dozen throwaway matmuls run during the
initial DMA wait to trip the PE HAM clock-gate to 2.4 GHz before the real
GEMM stream starts.
"""

import contextlib

import numpy as np
import ml_dtypes

import concourse.bass as bass
import concourse.mybir as mybir
import concourse.tile as tile
from concourse.bass_utils import run_bass_kernel_spmd

dt = mybir.dt
AF = mybir.ActivationFunctionType

H = 768          # model dim
B = 4            # batch
S = 2048         # sequence length
N_CORES = 8
QCHUNK = S * B // N_CORES        # 1024 query rows per core
HT = H // 128                    # 6 feature partition-tiles
KTILES = S // 128                # 16 key partition-tiles
QB = 512                         # q-block width (= PSUM bank, fp32)
NQB = QCHUNK // QB               # 2 q-blocks per core

FP8 = dt.float8e4
NP_FP8 = dt.np(FP8)
BF16 = dt.bfloat16
NP_BF16 = ml_dtypes.bfloat16
PMODE = mybir.MatmulPerfMode.DoubleRow

WQ_SCALE = 256.0                 # host premultiplier on Wq (incl 1/sqrt(H))
WK_SCALE = 64.0                  # host premultiplier on Wk
WO1_SCALE = 64.0                 # host premultiplier on Wo1
WO2_SCALE = 64.0                 # host premultiplier on Wo2


def build_kernel():
    nc = bass.Bass()

    qT_ext = nc.declare_dram_parameter("qT", [128, HT * QCHUNK], FP8, isOutput=False)
    kT_ext = nc.declare_dram_parameter("kT", [128, HT * S], FP8, isOutput=False)
    v_ext = nc.declare_dram_parameter("v", [128, KTILES * H], FP8, isOutput=False)
    vTb_ext = nc.declare_dram_parameter("vTb", [128, HT * QCHUNK], BF16,
                                        isOutput=False)
    w_ext = {
        name: nc.declare_dram_parameter(name, [128, HT * H], FP8, isOutput=False)
        for name in ("wqT", "wkT", "wo1T", "wo2T")
    }
    b_ext = nc.declare_dram_parameter("biases", [128, 3 * HT], dt.float32,
                                      isOutput=False)
    outT_ext = nc.declare_dram_parameter(
        "outT", [128, HT * QCHUNK], BF16, isOutput=True
    )

    with tile.TileContext(nc) as tc, nc.allow_low_precision(
        reason="fp8 matmul path is intentional; rel-err budget is 2e-2"
    ):
        _body(nc, tc, qT_ext, kT_ext, v_ext, vTb_ext, w_ext, b_ext, outT_ext)

    _split_multi_waits(nc)
    return nc


def _body(nc, tc, qT_ext, kT_ext, v_ext, vTb_ext, w_ext, b_ext, outT_ext):
    with contextlib.ExitStack() as ctx:
        const_pool = ctx.enter_context(tc.tile_pool(name="const", bufs=1))
        w_pool = ctx.enter_context(tc.tile_pool(name="w", bufs=1))
        act_pool = ctx.enter_context(tc.tile_pool(name="act", bufs=1))
        st_pool = ctx.enter_context(tc.tile_pool(name="st", bufs=1))
        out_pool = ctx.enter_context(tc.tile_pool(name="outs", bufs=3))
        # PSUM: 2 x [128,1024] rotating bank-pairs + 3 PV banks + 1 norm bank.
        ps_pair = ctx.enter_context(tc.tile_pool(name="ps_pair", bufs=2,
                                                 space="PSUM"))
        ps_one = ctx.enter_context(tc.tile_pool(name="ps_one", bufs=1,
                                                space="PSUM"))

        # ---- constants + PE warm-up (no DMA dependency: runs during the
        # input DMA head and trips the HAM clock gate to full rate) ----
        wu = const_pool.tile([128, 256], BF16, tag="warmup")
        nc.vector.memset(wu[:], 0.002)
        ones_f32 = const_pool.tile([128, 128], dt.float32, tag="ones_f32")
        nc.vector.memset(ones_f32[:], 1.0)
        ones_row = const_pool.tile([1, 128], dt.float32r, tag="ones_row")
        nc.vector.tensor_copy(ones_row[:], ones_f32[0:1, :])
        ones_col = const_pool.tile([128, 1], dt.float32r, tag="ones_col")
        nc.vector.tensor_copy(ones_col[:], ones_f32[:, 0:1])
        # fp8 ones pair for the qb1 denominator matmul: DR lhsT needs the
        # k-tile step to be a multiple of 16 bytes, so cols 0 and 16 of a
        # 32-wide tile are the two "rows" the AP actually reads.
        ones8 = const_pool.tile([128, 32], FP8, tag="ones8")
        nc.vector.memset(ones8[:], 1.0)
        ones8v = ones8[:].rearrange("p (t m) -> p t m", t=2)[:, :, 0:1]
        ln64 = const_pool.tile([128, 1], dt.float32, tag="ln64")
        nc.vector.memset(ln64[:], float(np.log(WO2_SCALE)))

        # ---- input DMAs in first-use order. The first-matmul chain (kT
        # block A, wkT chunk 0, biases, rest of wkT, kT block B) rides the
        # sync queue; ScalarE's queue opens with a dummy Ln activation that
        # pre-pays the ~2.7us ACT table load while those DMAs fly (Ln selects
        # natural_log_exp_and_others, which also holds Exp/Relu/Identity/
        # Copy, so this is the only table load in the kernel). ----
        # wkT chunks ride the scalar HWDGE queue (parallel with kT on sync),
        # issued BEFORE the dummy Ln act so the table load doesn't delay them
        w_sb = {
            name: w_pool.tile([128, HT * H], FP8, tag=name, name=f"w_{name}")
            for name in ("wqT", "wkT", "wo1T", "wo2T")
        }
        for c0, c1 in ((0, 1), (1, 2), (2, 3), (3, HT)):
            nc.scalar.dma_start(w_sb["wkT"][:, c0 * HT * 128: c1 * HT * 128],
                                w_ext["wkT"][:, c0 * HT * 128: c1 * HT * 128])
        actwarm = const_pool.tile([1, 2], dt.float32, tag="actwarm")
        nc.scalar.activation(actwarm[:], ones_f32[0:1, 0:2], AF.Ln)

        # kT block A lands in contraction order (ht tiles 0-3 of both
        # sub-blocks first) so the first projection matmuls start sooner
        kT_in = act_pool.tile([128, HT * S], FP8, tag="kT_in")
        C2 = 4 * 512
        for a, b in ((0, C2), (3072, 3072 + C2), (C2, 3072), (3072 + C2, 6144)):
            nc.sync.dma_start(kT_in[:, a:b], kT_ext[:, a:b])
        bias_sb = const_pool.tile([128, 3 * HT], dt.float32, tag="biases")
        nc.sync.dma_start(bias_sb[:], b_ext[:])
        biases = {name: bias_sb[:, i * HT:(i + 1) * HT]
                  for i, name in enumerate(("bq", "bk", "bo1"))}
        nc.sync.dma_start(kT_in[:, HT * QB * 2:], kT_ext[:, HT * QB * 2:])
        nc.scalar.dma_start(w_sb["wqT"][:], w_ext["wqT"][:])
        qT_in = act_pool.tile([128, HT * QCHUNK], FP8, tag="qT_in")
        nc.sync.dma_start(qT_in[:], qT_ext[:])
        # v / vTb / MLP weights aren't needed until 52-95us, but issuing
        # their DMAs now would steal HBM bandwidth from the critical K/Q
        # inputs (all 16 DMA engines round-robin). Their triggers are
        # emitted below AFTER the projection activations, so scalar-queue
        # position delays them past the head.
        v_sb = act_pool.tile([128, KTILES * H], FP8, tag="v_in")
        vTb_in = act_pool.tile([128, HT * QCHUNK], BF16, tag="vTb_in")

        # warm-up matmuls (~1.5us of junk PE work into the norm bank)
        for i in range(7):
            ps_wu = ps_one.tile([128, QB], dt.float32, tag="norm",
                                name=f"wu{i}")
            nc.tensor.matmul(ps_wu[:, :256], wu[:, :128], wu[:],
                             start=True, stop=True)

        def w3(name):
            return w_sb[name][:].rearrange("p (o t m) -> p (o t) m", o=HT, t=HT)

        # ---- K/Q projections: jo-contraction into [128,1024] bank-pairs,
        # one wide ScalarE activation per (ot, block-pair) ----
        def project(wname, bias, scale, x_in, nblocks, out_sb, out_col, tag,
                    head_split=False):
            wv = w3(wname)
            xv = x_in[:].rearrange("p (n t q) -> p n t q", n=nblocks, t=HT)
            head_banks = {(0, 0): "pvw0", (0, 1): "pvw1",
                          (1, 0): "pvw2", (1, 1): "norm"}
            deferred = []
            for nb2 in range(nblocks // 2):
                for ot in range(HT):
                    # the very first evacuations ride 4 idle single banks so
                    # the act-latency ramp doesn't stall the bank-pair pool
                    split = head_split and nb2 == 0 and ot < 2
                    if split:
                        tiles = [ps_one.tile([128, QB], dt.float32,
                                             tag=head_banks[(ot, h)],
                                             name=f"ps_{tag}h_{ot}_{h}")
                                 for h in range(2)]
                        tgt = lambda h: tiles[h][:]
                    else:
                        pair = ps_pair.tile([128, 2 * QB], dt.float32,
                                            tag="pair",
                                            name=f"ps_{tag}_{nb2}_{ot}")
                        tgt = lambda h: pair[:, h * QB:(h + 1) * QB]
                    for jo in range(HT // 2):
                        for h in range(2):
                            nc.tensor.matmul(
                                tgt(h),
                                wv[:, ot * HT + 2 * jo: ot * HT + 2 * jo + 2, :],
                                xv[:, 2 * nb2 + h, 2 * jo: 2 * jo + 2, :],
                                start=(jo == 0),
                                stop=(jo == HT // 2 - 1),
                                perf_mode=PMODE,
                            )
                    c0 = out_col(ot, nb2)
                    if split:
                        # defer these evacuations: their banks aren't needed
                        # until PV, and emitting them later keeps the first
                        # NEEDED act (ot2's, gating ot4) at the queue front
                        deferred.append((c0, tiles, bias[:, ot: ot + 1]))
                    else:
                        nc.scalar.activation(
                            out_sb[:, c0: c0 + 2 * QB], pair[:], AF.Identity,
                            bias=bias[:, ot: ot + 1], scale=scale,
                        )
                        if deferred and ot == 3:
                            for d0, dtiles, db in deferred:
                                for h in range(2):
                                    nc.scalar.activation(
                                        out_sb[:, d0 + h * QB:
                                               d0 + (h + 1) * QB],
                                        dtiles[h][:], AF.Identity,
                                        bias=db, scale=scale,
                                    )
                            deferred = []

        KT = act_pool.tile([128, HT * S], FP8, tag="KT", name="KT_full")
        project("wkT", biases["bk"], 1.0 / WK_SCALE, kT_in, 4, KT,
                lambda ot, nb2: ot * S + nb2 * 2 * QB, "KT", head_split=True)
        nc.scalar.dma_start(v_sb[:], v_ext[:])
        QT = act_pool.tile([128, HT * QCHUNK], FP8, tag="QT", name="proj_QT")
        project("wqT", biases["bq"], 1.0 / WQ_SCALE, qT_in, 2, QT,
                lambda ot, nb2: ot * QCHUNK + nb2 * 2 * QB, "QT")
        nc.scalar.dma_start(vTb_in[:], vTb_ext[:])
        nc.scalar.dma_start(w_sb["wo1T"][:], w_ext["wo1T"][:])
        nc.scalar.dma_start(w_sb["wo2T"][:], w_ext["wo2T"][:])
        KT3 = KT[:].rearrange("p (t k) -> p t k", t=HT)
        QT3 = QT[:].rearrange("p (t q) -> p t q", t=HT)

        def vpair(jk, ht):
            """lhsT [128, 2, 128]: k-tile pair (2jk, 2jk+1), h-tile ht."""
            return (v_sb[:].rearrange("p (t h) -> p t h", t=KTILES)
                    [:, 2 * jk: 2 * jk + 2, ht * 128:(ht + 1) * 128])

        # ---- attention + MLP, software-pipelined across q-blocks ----
        state = {}

        def den_mm(qb, p8):
            """qb1 path: accumulate exp-pair p8 into the [1,512] rowsum via
            a ones matmul (contraction over 128 partitions x 2 k-tiles)."""
            st = state[qb]
            rhs8 = st["expT"][:].rearrange("p (j t q) -> p j t q",
                                           j=KTILES // 2, t=2)
            nc.tensor.matmul(
                st["ps_den"][0:1, :], ones8v, rhs8[:, p8],
                start=(p8 == 0), stop=(p8 == KTILES // 2 - 1),
                perf_mode=PMODE,
            )

        def phase_scores(qb):
            """scores^T + exp, two k-tiles per PSUM bank-pair / ACTIVATE.
            Denominator: qb0 accumulates on the (idle there) DVE as
            incremental [128,1024] adds; qb1's scores window already has
            weighted-qb0 DVE work, so its denominator rides the PE as tiny
            ones-matmuls trailing one pair behind the exps."""
            q0 = qb * QB
            expT = st_pool.tile([128, KTILES * QB], FP8, tag=f"expT{qb}",
                                name=f"expT{qb}")
            state[qb] = {"expT": expT}
            if qb == 0:
                acc = st_pool.tile([128, 2 * QB], BF16, tag="acc0")
            else:
                state[qb]["ps_den"] = ps_one.tile(
                    [128, QB], dt.float32, tag="norm", name="ps_den1")
            for p8 in range(KTILES // 2):
                pair = ps_pair.tile([128, 2 * QB], dt.float32, tag="pair",
                                    name=f"ps_s_{qb}_{p8}")
                for half in range(2):
                    kt = 2 * p8 + half
                    for jo in range(HT // 2):
                        nc.tensor.matmul(
                            pair[:, half * QB:(half + 1) * QB],
                            KT3[:, 2 * jo: 2 * jo + 2, kt * 128:(kt + 1) * 128],
                            QT3[:, 2 * jo: 2 * jo + 2, q0: q0 + QB],
                            start=(jo == 0),
                            stop=(jo == HT // 2 - 1),
                            perf_mode=PMODE,
                        )
                sl = expT[:, p8 * 2 * QB:(p8 + 1) * 2 * QB]
                nc.scalar.activation(sl, pair[:], AF.Exp)
                if qb == 0:
                    if p8 == 0:
                        nc.vector.tensor_copy(acc[:], sl)
                    else:
                        nc.vector.tensor_add(acc[:], acc[:], sl)
                elif p8 >= 1:
                    den_mm(qb, p8 - 1)
            if qb == 0:
                sum_part = st_pool.tile([128, QB], dt.float32r, tag="sump0")
                nc.vector.tensor_add(sum_part[:], acc[:, :QB], acc[:, QB:])
                state[qb]["sum_part"] = sum_part

        def phase_norm_ln(qb):
            """Partition-reduce the rowsum (qb0: one ones-matmul on the DVE
            partial; qb1: close the PE den group), then 1/rowsum =
            exp(-ln(rowsum)) on ScalarE: no DVE reciprocal."""
            st = state[qb]
            if qb == 0:
                ps_den = ps_one.tile([128, QB], dt.float32, tag="norm",
                                     name="ps_den0")
                nc.tensor.matmul(ps_den[0:1, :], ones_col[:],
                                 st["sum_part"][:], start=True, stop=True)
            else:
                den_mm(qb, KTILES // 2 - 1)
                ps_den = st["ps_den"]
            logsum = st_pool.tile([1, QB], dt.float32r, tag="logsum",
                                  name=f"logsum{qb}")
            nc.scalar.activation(logsum[:], ps_den[0:1, :], AF.Ln)
            st["logsum"] = logsum

        def phase_norm_bcast(qb):
            """bcast64 = exp(ln64 - ln(rowsum)) = 64/rowsum: the x64 keeps
            the weighted residual in the same scale as the x64 Wo2 PSUM so
            the MLP output evacuates in ONE fused DVE op."""
            st = state[qb]
            ps_b = ps_one.tile([128, QB], dt.float32, tag="norm",
                               name=f"ps_b{qb}")
            nc.tensor.matmul(ps_b[:], ones_row[:], st["logsum"][:],
                             start=True, stop=True)
            bcast = st_pool.tile([128, QB], dt.float32, tag="bcast",
                                 name=f"bcast{qb}")
            nc.scalar.activation(bcast[:], ps_b[:], AF.Exp, scale=-1.0)
            st["bcast"] = bcast

        def phase_pv_half(qb, half):
            """PV for 3 h-tiles over all 16 k-tiles; norm chain of this
            q-block interleaves under half 0."""
            st = state[qb]
            rhs8 = st["expT"][:].rearrange("p (j t q) -> p j t q",
                                           j=KTILES // 2, t=2)
            ps_w = [ps_one.tile([128, QB], dt.float32, tag=f"pvw{i}",
                                name=f"pvw{i}_{qb}_{half}")
                    for i in range(3)]
            if half == 0:
                sched = [(jk, i) for jk in range(KTILES // 2) for i in range(3)]
            else:
                # skewed wavefront: bank i starts i waves late, so the first
                # matmul of each bank lands just after the previous half's
                # weighted-mul releases that bank (no lump wait on DVE)
                sched = [(w - i, i) for w in range(KTILES // 2 + 2)
                         for i in range(3) if 0 <= w - i < KTILES // 2]
            for n, (jk, i) in enumerate(sched):
                if half == 0 and (jk, i) == (1, 0):
                    phase_norm_ln(qb)
                if half == 0 and (jk, i) == (3, 0):
                    phase_norm_bcast(qb)
                nc.tensor.matmul(
                    ps_w[i][:],
                    vpair(jk, 3 * half + i),
                    rhs8[:, jk],
                    start=(jk == 0),
                    stop=(jk == KTILES // 2 - 1),
                    perf_mode=PMODE,
                )
            st[f"ps_w{half}"] = ps_w

        def phase_weighted_half(qb, half):
            """w = PV/rowsum + (value^T + bo2); bf16 residual + fp8 GEMM copy."""
            st = state[qb]
            ps_w = st[f"ps_w{half}"]
            if "wr" not in st:
                st["wr"] = st_pool.tile([128, HT * QB], BF16, tag=f"wr{qb}",
                                        name=f"wr{qb}")
                st["w8"] = st_pool.tile([128, HT * QB], FP8, tag=f"w8_{qb}",
                                        name=f"w8_{qb}")
            wr, w8 = st["wr"], st["w8"]
            for i in range(3):
                ht = 3 * half + i
                c0 = ht * QB
                tmp = out_pool.tile([128, QB], dt.float32, tag="wtmp",
                                    name=f"wtmp_{qb}_{ht}")
                nc.vector.tensor_mul(tmp[:], ps_w[i][:], st["bcast"][:])
                nc.vector.tensor_add(
                    wr[:, c0: c0 + QB], tmp[:],
                    vTb_in[:, qb * HT * QB + c0: qb * HT * QB + c0 + QB],
                )
                if half == 1:
                    # half-1 casts' queue slots on ScalarE would sit in
                    # front of (and so gate) the next phase's activations
                    # under the coarse per-engine semaphores -- DVE instead
                    nc.vector.tensor_copy(w8[:, c0: c0 + QB],
                                          wr[:, c0: c0 + QB])
                else:
                    nc.scalar.copy(w8[:, c0: c0 + QB], wr[:, c0: c0 + QB])

        def phase_mlp_h1(qb):
            st = state[qb]
            w8v = st["w8"][:].rearrange("p (t q) -> p t q", t=HT)
            wv = w3("wo1T")
            h1 = st_pool.tile([128, HT * QB], FP8, tag=f"h1_{qb}",
                              name=f"h1T{qb}")
            for otp in range(HT // 2):
                pair = ps_pair.tile([128, 2 * QB], dt.float32, tag="pair",
                                    name=f"ps_h1_{qb}_{otp}")
                for h in range(2):
                    ot = 2 * otp + h
                    for jo in range(HT // 2):
                        nc.tensor.matmul(
                            pair[:, h * QB:(h + 1) * QB],
                            wv[:, ot * HT + 2 * jo: ot * HT + 2 * jo + 2, :],
                            w8v[:, 2 * jo: 2 * jo + 2, :],
                            start=(jo == 0),
                            stop=(jo == HT // 2 - 1),
                            perf_mode=PMODE,
                        )
                for h in range(2):
                    ot = 2 * otp + h
                    nc.scalar.activation(
                        h1[:, ot * QB:(ot + 1) * QB],
                        pair[:, h * QB:(h + 1) * QB],
                        AF.Relu, bias=biases["bo1"][:, ot: ot + 1],
                        scale=1.0 / WO1_SCALE,
                    )
            st["h1"] = h1

        def phase_mlp_out(qb):
            """out = h1 @ Wo2^T + (w + bo2): PSUM + residual in one DVE add,
            straight to bf16 DMA staging."""
            st = state[qb]
            h1v = st["h1"][:].rearrange("p (t q) -> p t q", t=HT)
            wv = w3("wo2T")
            for otp in range(HT // 2):
                pair = ps_pair.tile([128, 2 * QB], dt.float32, tag="pair",
                                    name=f"ps_o_{qb}_{otp}")
                for h in range(2):
                    ot = 2 * otp + h
                    for jo in range(HT // 2):
                        nc.tensor.matmul(
                            pair[:, h * QB:(h + 1) * QB],
                            wv[:, ot * HT + 2 * jo: ot * HT + 2 * jo + 2, :],
                            h1v[:, 2 * jo: 2 * jo + 2, :],
                            start=(jo == 0),
                            stop=(jo == HT // 2 - 1),
                            perf_mode=PMODE,
                        )
                o_mid = out_pool.tile([128, 2 * QB], BF16, tag="o_mid",
                                      name=f"omid_{qb}_{otp}")
                o_sb = out_pool.tile([128, 2 * QB], BF16, tag="outT_blk",
                                     name=f"outT_{qb}_{otp}")
                c0 = (qb * HT + otp * 2) * QB
                # the very last block streams out in 512-wide chunks so the
                # act -> add -> DMA tail pipeline overlaps
                nchunk = 2 if (qb, otp) == (1, HT // 2 - 1) else 1
                for ch in range(nchunk):
                    w = 2 * QB // nchunk
                    sl = slice(ch * w, (ch + 1) * w)
                    nc.scalar.activation(o_mid[:, sl], pair[:, sl], AF.Identity,
                                         scale=1.0 / WO2_SCALE)
                    nc.vector.tensor_add(
                        o_sb[:, sl], o_mid[:, sl],
                        st["wr"][:, otp * 2 * QB + ch * w:
                                 otp * 2 * QB + (ch + 1) * w],
                    )
                    nc.sync.dma_start(
                        outT_ext[:, c0 + ch * w: c0 + (ch + 1) * w],
                        o_sb[:, sl],
                    )

        # software pipeline: DVE/ScalarE chains (norm, weighted, h1-acts) are
        # always covered by an independent PE phase emitted around them
        phase_scores(0)
        phase_pv_half(0, 0)
        phase_weighted_half(0, 0)
        phase_pv_half(0, 1)
        phase_weighted_half(0, 1)
        phase_scores(1)
        phase_mlp_h1(0)
        phase_pv_half(1, 0)
        phase_weighted_half(1, 0)
        phase_pv_half(1, 1)
        phase_weighted_half(1, 1)
        phase_mlp_out(0)
        phase_mlp_h1(1)
        phase_mlp_out(1)


# ---- host-side shard packing ----

def _tile_rows(a):
    """[T*128, N] -> [128, T*N]: partition-tiled T-layout, contiguous DMA."""
    t = a.shape[0] // 128
    return a.reshape(t, 128, a.shape[1]).transpose(1, 0, 2).reshape(128, -1)


def _tile_weight(w):
    """W^T [768h, 768o] -> [128, (ot, ht, 128)]: o-major packed lhsT tiles."""
    x = w.reshape(HT, 128, HT, 128)          # [ht, p, ot, o128]
    return x.transpose(1, 2, 0, 3).reshape(128, -1)


def _tile_rows_blocked(a, qb):
    """[768, NB*qb] -> [128, NB*(6*qb)]: per-block ht-major packing."""
    nb = a.shape[1] // qb
    x = a.reshape(HT, 128, nb, qb).transpose(1, 2, 0, 3)
    return x.reshape(128, -1)


def shard_inputs(query, key, value, Wq, bq, Wk, bk, Wo1, bo1, Wo2, bo2):
    """Full inputs -> per-core in_maps (host packing, fp8 cast, scale folds)."""
    scale = np.float32(1.0 / np.sqrt(np.float32(H)))

    def c8(x):
        return np.ascontiguousarray(
            np.clip(np.asarray(x, np.float32), -240, 240).astype(NP_FP8))

    def cb(x):
        return np.ascontiguousarray(np.asarray(x, np.float32).astype(NP_BF16))

    def cf(x):
        return np.ascontiguousarray(x.astype(np.float32))

    bo1p = bo1 - Wo1 @ bo2           # corrects for the +bo2 folded into w'
    shared = {
        "wqT": c8(_tile_weight(Wq.T * (scale * WQ_SCALE))),
        "wkT": c8(_tile_weight(Wk.T * WK_SCALE)),
        "wo1T": c8(_tile_weight(Wo1.T * WO1_SCALE)),
        "wo2T": c8(_tile_weight(Wo2.T * WO2_SCALE)),
        "biases": cf(np.concatenate([
            (bq * scale).reshape(HT, 128).T, bk.reshape(HT, 128).T,
            np.asarray(bo1p).reshape(HT, 128).T], axis=1)),
    }
    in_maps = []
    for core in range(N_CORES):
        b, half = divmod(core, 2)
        r0 = half * QCHUNK
        vTb = np.asarray(value[b]).T + np.asarray(bo2)[:, None]
        in_maps.append({
            "qT": c8(_tile_rows_blocked(query[b].T[:, r0: r0 + QCHUNK], QB)),
            "kT": c8(_tile_rows_blocked(key[b].T, QB)),
            "v": c8(_tile_rows(np.asarray(value[b]))),
            "vTb": cb(_tile_rows_blocked(vTb[:, r0: r0 + QCHUNK], QB)),
            **shared,
        })
    return in_maps


def gather_outputs(results):
    """Per-core outT [128, NQB*HT*QB] bf16 -> full [B, S, H] fp32."""
    out = np.empty((B, S, H), dtype=np.float32)
    for core in range(N_CORES):
        b, half = divmod(core, 2)
        r0 = half * QCHUNK
        buf = results[core]["outT"].reshape(128, NQB, HT, QB)
        # out[q0+qb*QB+n, ot*128+p] = buf[p, qb, ot, n]
        out[b, r0: r0 + QCHUNK] = (
            buf.transpose(1, 3, 2, 0).reshape(QCHUNK, H).astype(np.float32)
        )
    return out


def run(inputs, trace=False):
    nc = build_kernel()
    in_maps = shard_inputs(**{k: np.asarray(v) for k, v in inputs.items()})
    res = run_bass_kernel_spmd(nc, in_maps, list(range(N_CORES)), trace=trace)
    return gather_outputs(res.results), res


def _split_multi_waits(nc):
    """Workaround for this container's walrus rejecting instructions that
    carry more than one semaphore wait ("Too many sync wait commands"):
    hoist N-1 waits onto fresh single-wait same-engine InstNoOp instructions
    inserted immediately before the instruction. Engine streams execute the
    block's per-engine subsequence in order, so blocking on the nops first is
    semantically identical to one multi-wait instruction."""
    for f in nc.m.functions:
        for bb in f.blocks:
            insts = list(bb.instructions)
            out = []
            changed = False
            for inst in insts:
                si = inst.sync_info
                waits = list(si.on_wait) if si is not None and si.on_wait else []
                if len(waits) > 1:
                    changed = True
                    for w in waits[:-1]:
                        nop = mybir.InstNoOp(
                            name=nc.get_next_instruction_name(), ins=[], outs=[]
                        )
                        nop.engine = inst.engine
                        nop.sync_info = mybir.SyncInfo(on_wait=[w], on_update=[])
                        out.append(nop)
                    si.on_wait = waits[-1:]
                    inst.sync_info = si
                out.append(inst)
            if changed:
                bb.instructions = out


def kernel(**inputs):
    """Entry point: full (unsharded) numpy inputs -> full [B, S, H] output."""
    out, _ = run(inputs, trace=False)
    return out
